# revision 1
# baseline (speedup 1.0000x reference)
"""Trainium2 Bass kernel for BottleneckedEnsembleAttention.

Sharding: 8 cores, core c handles heads [2c, 2c+1] for both batches
(4 independent (b, head) attention problems per core).

Per (b, h) on-device pipeline (all matmuls in float32r, 1 cycle/col):
  1. load X [2048, 1024] natural, PE-transpose to X^T (8 tiles [128, 2048])
  2. qk pass:   psum = [Wq|Wk]^T X^T        -> [128, 2048] (rows 0-63 q^T, 64-127 k^T)
     qkrot pass: psum = [Wq'|Wk']^T X^T     (W' = rotate-half permuted/negated weights)
     RoPE: QKroped = psum_qk * cosT + psum_rot * sinT   (scale folded into q-half of tables)
  3. v^T pass, PE-transpose to v natural [s, 64], append ones column -> [s, 65]
  4. per t-chunk (512 cols): scores^T[s-tile, t] = k_roped lhsT @ q_roped rhs
     (consecutive s-tiles row-packed into array halves via partition bases),
     exp via ACT (per-partition bias = -1e30 for inactive s), causal via
     block skip + GPSIMD multiply with per-diagonal mask tiles,
     att^T[u, t] accumulated over s-tiles (extra ones-column in v gives the
     softmax denominator for free in row 64),
     o_proj: out[t-tile, 1024] = att^T lhsT @ Wo rhs (row-packed pairs),
     final scale by active[t] / denom[t] during PSUM eviction.

The emission is software-pipelined across the 4 (b, h) problems: the next
pair's X load + PE transposes are interleaved as PE gap-filler inside the
current pair's attention chunks, and each chunk's o_proj/store is deferred
one chunk so the PE never waits on eviction chains.

Host precomputes (numpy): YaRN cos/sin tables (transposed layout, scale and
rotate-half signs folded in), packed/rotated weights, active-mask bias rows,
diagonal causal masks, identity.
"""

import math
from contextlib import ExitStack

import numpy as np

import concourse.bass as bass
import concourse.mybir as mybir
import concourse.tile as tile
from concourse import bacc
from concourse.bass_utils import run_bass_kernel_spmd

# model constants (must match reference.py)
HIDDEN = 1024
HEADS = 16
HEAD_DIM = 64
THETA = 10000.0
TRAIN_LEN = 2048
SCALE = 4.0
ALPHA = 1.0
BETA = 32.0
B, T = 2, 2048

NCORES = 8
HPC = HEADS // NCORES  # heads per core = 2

F32 = mybir.dt.float32
F32R = mybir.dt.float32r

NEG_BIG = -1.0e30
DENOM_EPS = 1.0e-30

NT = T // 128   # 16 t-tiles of 128
NC4 = T // 512  # 4 chunks of 512
ND = HIDDEN // 128  # 8 d-chunks


def _yarn_inv_freq_and_mscale():
    half = HEAD_DIM // 2
    pos_freqs = THETA ** (np.arange(half, dtype=np.float32) * 2.0 / HEAD_DIM)
    inv_freq_extra = (1.0 / pos_freqs).astype(np.float32)
    inv_freq_inter = (1.0 / (SCALE * pos_freqs)).astype(np.float32)

    def find_dim(num_rot):
        return (HEAD_DIM * math.log(TRAIN_LEN / (num_rot * 2.0 * math.pi))) / (
            2.0 * math.log(THETA)
        )

    low = max(math.floor(find_dim(BETA)), 0)
    high = min(math.ceil(find_dim(ALPHA)), half - 1)
    ramp = np.clip(
        (np.arange(half, dtype=np.float32) - low) / max(high - low, 1e-3), 0.0, 1.0
    ).astype(np.float32)
    extrap = (1.0 - ramp).astype(np.float32)
    inv_freq = inv_freq_inter * (1.0 - extrap) + inv_freq_extra * extrap
    mscale = 0.1 * math.log(SCALE) + 1.0 if SCALE > 1.0 else 1.0
    return inv_freq.astype(np.float32), np.float32(mscale)


def _rot_w(w):
    # w: (h, HIDDEN, 64). rot_half(X@w) == X @ rot_w(w)
    return np.concatenate([-w[..., 32:], w[..., :32]], axis=-1)


def _host_prep(inputs):
    x = np.ascontiguousarray(inputs["packed_embeddings"], dtype=np.float32)
    pos = np.asarray(inputs["position_ids"])
    act = np.asarray(inputs["active_mask"])
    wq = np.asarray(inputs["q_proj"], dtype=np.float32)
    wk = np.asarray(inputs["k_proj"], dtype=np.float32)
    wv = np.asarray(inputs["v_proj"], dtype=np.float32)
    wo = np.asarray(inputs["o_proj"], dtype=np.float32)

    inv_freq, mscale = _yarn_inv_freq_and_mscale()
    scale = np.float32(mscale / math.sqrt(HEAD_DIM))

    ang = pos.astype(np.float32)[..., None] * inv_freq  # (B, L, T, 32)
    cos32 = np.cos(ang).astype(np.float32)
    sin32 = np.sin(ang).astype(np.float32)
    cos64 = np.concatenate([cos32, cos32], axis=-1)  # (B, L, T, 64)
    sin64 = np.concatenate([sin32, sin32], axis=-1)
    # transposed tables [B, L, 128, T]: rows 0-63 for q (scale folded), 64-127 for k
    cosT = np.concatenate([cos64 * scale, cos64], axis=-1).transpose(0, 1, 3, 2)
    sinT = np.concatenate([sin64 * scale, sin64], axis=-1).transpose(0, 1, 3, 2)
    cosT = np.ascontiguousarray(cosT, dtype=np.float32)
    sinT = np.ascontiguousarray(sinT, dtype=np.float32)

    wqk = np.ascontiguousarray(np.concatenate([wq, wk], axis=-1))  # (L, 1024, 128)
    wqkr = np.ascontiguousarray(np.concatenate([_rot_w(wq), _rot_w(wk)], axis=-1))
    wv = np.ascontiguousarray(wv)  # (L, 1024, 64)
    wo = np.ascontiguousarray(wo)  # (L, 64, 1024)

    actf = act.astype(np.float32)  # (B, L, T)
    # bias rows for exp: 0 where active, -1e30 where inactive; layout [B,L,128,NT]
    actb = ((actf - 1.0) * (-NEG_BIG)).reshape(B, HEADS, NT, 128).transpose(0, 1, 3, 2)
    actb = np.ascontiguousarray(actb, dtype=np.float32)
    act01 = np.ascontiguousarray(
        actf.reshape(B, HEADS, NT, 128).transpose(0, 1, 3, 2), dtype=np.float32
    )  # [B, L, 128, NT]

    # diagonal-chunk masks: variant kd zeroes cols < kd*128, upper-tri on its
    # own 128-block, ones after
    dmask = np.ones((4, 128, 512), dtype=np.float32)
    tri = np.triu(np.ones((128, 128), dtype=np.float32))
    for kd in range(4):
        dmask[kd, :, :kd * 128] = 0.0
        dmask[kd, :, kd * 128:(kd + 1) * 128] = tri
    dmask = np.ascontiguousarray(dmask)
    vones = np.zeros((128, 2), dtype=np.float32)
    vones[:, 0] = 1.0
    ident = np.ascontiguousarray(np.eye(128, dtype=np.float32))
    return x, cosT, sinT, wqk, wqkr, wv, wo, actb, act01, dmask, vones, ident


def _build_program(repeats=1):
    nc = bacc.Bacc("TRN2", target_bir_lowering=False, debug=False)

    x_d = nc.declare_dram_parameter("x", [B, HPC, T, HIDDEN], F32R, isOutput=False)
    cos_d = nc.declare_dram_parameter("cos", [B, HPC, 128, T], F32, isOutput=False)
    sin_d = nc.declare_dram_parameter("sin", [B, HPC, 128, T], F32, isOutput=False)
    wqk_d = nc.declare_dram_parameter("wqk", [HPC, HIDDEN, 128], F32R, isOutput=False)
    wqkr_d = nc.declare_dram_parameter("wqkr", [HPC, HIDDEN, 128], F32R, isOutput=False)
    wv_d = nc.declare_dram_parameter("wv", [HPC, HIDDEN, HEAD_DIM], F32R, isOutput=False)
    wo_d = nc.declare_dram_parameter("wo", [HPC, HEAD_DIM, HIDDEN], F32R, isOutput=False)
    actb_d = nc.declare_dram_parameter("actb", [B, HPC, 128, NT], F32, isOutput=False)
    act01_d = nc.declare_dram_parameter("act01", [B, HPC, 128, NT], F32, isOutput=False)
    dmask_d = nc.declare_dram_parameter("dmask", [4, 128, 512], F32, isOutput=False)
    vones_d = nc.declare_dram_parameter("vones", [128, 2], F32R, isOutput=False)
    ident_d = nc.declare_dram_parameter("ident", [128, 128], F32R, isOutput=False)
    out_d = nc.declare_dram_parameter("out", [B, HPC, T, HIDDEN], F32, isOutput=True)

    with ExitStack() as ctx:
        tc = ctx.enter_context(tile.TileContext(nc))
        _emit(ctx, tc, nc, x_d, cos_d, sin_d, wqk_d, wqkr_d, wv_d, wo_d,
              actb_d, act01_d, dmask_d, vones_d, ident_d, out_d,
              repeats=repeats)
    nc.compile()
    return nc


def _emit(ctx, tc, nc, x_d, cos_d, sin_d, wqk_d, wqkr_d, wv_d, wo_d,
          actb_d, act01_d, dmask_d, vones_d, ident_d, out_d, repeats=1):
    # ---- pools ----
    consts = ctx.enter_context(tc.tile_pool(name="consts", bufs=1))
    wpool = ctx.enter_context(tc.tile_pool(name="wpool", bufs=1))
    xnp = ctx.enter_context(tc.tile_pool(name="xn", bufs=8))
    xtp = ctx.enter_context(tc.tile_pool(name="xt", bufs=1))
    qkp = ctx.enter_context(tc.tile_pool(name="qk", bufs=1))
    krsp = ctx.enter_context(tc.tile_pool(name="krs", bufs=1))
    tmpp = ctx.enter_context(tc.tile_pool(name="tmps", bufs=2))
    vtp = ctx.enter_context(tc.tile_pool(name="vt", bufs=1))
    vnp = ctx.enter_context(tc.tile_pool(name="vn", bufs=2))
    probp = ctx.enter_context(tc.tile_pool(name="prob", bufs=4))
    attp = ctx.enter_context(tc.tile_pool(name="att", bufs=2))
    rap = ctx.enter_context(tc.tile_pool(name="ra", bufs=2))
    outp = ctx.enter_context(tc.tile_pool(name="outsb", bufs=2))
    cssp = ctx.enter_context(tc.tile_pool(name="css", bufs=1))
    abp = ctx.enter_context(tc.tile_pool(name="ab", bufs=2))

    psum = ctx.enter_context(tc.tile_pool(name="psum", bufs=2, space="PSUM"))
    psum_sc = ctx.enter_context(tc.tile_pool(name="psum_sc", bufs=2, space="PSUM"))
    psum_att = ctx.enter_context(tc.tile_pool(name="psum_att", bufs=1, space="PSUM"))
    psum_dn = ctx.enter_context(tc.tile_pool(name="psum_dn", bufs=1, space="PSUM"))
    psum_o = ctx.enter_context(tc.tile_pool(name="psum_o", bufs=2, space="PSUM"))

    # ---- constants / weights (once) ----
    ident_sb = consts.tile([128, 128], F32R)
    nc.sync.dma_start(out=ident_sb, in_=ident_d[:, :])
    dmask_sb = consts.tile([128, 4, 512], F32)
    nc.sync.dma_start(out=dmask_sb, in_=dmask_d.rearrange("k p n -> p k n"))
    vones_sb = consts.tile([128, 2], F32R)
    nc.sync.dma_start(out=vones_sb, in_=vones_d[:, :])
    ones_sb = consts.tile([128, 1], F32)
    nc.vector.memset(ones_sb, 1.0)

    pairs = []
    for _rep in range(repeats):
        for b in range(B):
            for h in range(HPC):
                pairs.append((b, h))
    n_pairs = len(pairs)
    st = {}       # per-pair-idx state
    pending = []  # deferred chunk finishers

    # ---------- phase emitters ----------
    def emit_tables(idx):
        b, h = pairs[idx]
        s = st[idx] = {}
        s["cos"] = cssp.tile([128, T], F32, tag="cos", name="cos_sb")
        nc.sync.dma_start(out=s["cos"], in_=cos_d[b, h])
        s["sin"] = cssp.tile([128, T], F32, tag="sin", name="sin_sb")
        nc.sync.dma_start(out=s["sin"], in_=sin_d[b, h])
        s["actb"] = abp.tile([128, NT], F32, tag="actb", name="actb_sb")
        nc.sync.dma_start(out=s["actb"], in_=actb_d[b, h])
        s["act01"] = abp.tile([128, NT], F32, tag="act01", name="act01_sb")
        nc.sync.dma_start(out=s["act01"], in_=act01_d[b, h])
        s["xt"] = [xtp.tile([128, T], F32R, tag=f"xt{dc}", name=f"xt{dc}")
                   for dc in range(ND)]
        t_qk = wpool.tile([128, ND, 128], F32R, tag="wqk", name="t_qk")
        nc.sync.dma_start(out=t_qk, in_=wqk_d[h].rearrange("(c p) m -> p c m", p=128))
        t_qkr = wpool.tile([128, ND, 128], F32R, tag="wqkr", name="t_qkr")
        nc.sync.dma_start(out=t_qkr, in_=wqkr_d[h].rearrange("(c p) m -> p c m", p=128))
        t_v = wpool.tile([128, ND, HEAD_DIM], F32R, tag="wv", name="t_v")
        nc.sync.dma_start(out=t_v, in_=wv_d[h].rearrange("(c p) m -> p c m", p=128))
        t_o = wpool.tile([128, HIDDEN], F32R, tag="wo", name="t_o")
        nc.sync.dma_start(out=t_o[0:HEAD_DIM, :], in_=wo_d[h])
        nc.sync.dma_start(out=t_o[HEAD_DIM:2 * HEAD_DIM, :], in_=wo_d[h])
        s["wqk"], s["wqkr"], s["wv"], s["wo"] = t_qk, t_qkr, t_v, t_o

    def emit_a_steps(idx):
        # X load + transpose for all 16 t-tiles; one yield per (group, d-chunk)
        # step. Groups' xn DMAs are issued one group ahead (xn bufs=8).
        b, h = pairs[idx]
        s = st[idx]

        def load_group(tg):
            tiles = []
            for k in range(4):
                ti = tg * 4 + k
                xn = xnp.tile([128, HIDDEN], F32R, tag="xn", name="xn")
                nc.sync.dma_start(out=xn,
                                  in_=x_d[b, h, ti * 128:(ti + 1) * 128, :])
                tiles.append(xn)
            return tiles

        def gen():
            nxt = load_group(0)
            for tg in range(NT // 4):
                cur = nxt
                if tg + 1 < NT // 4:
                    nxt = load_group(tg + 1)
                for dc in range(ND):
                    ptr = psum.tile([128, 512], F32, tag="proj", name="ptr")
                    for k in range(4):
                        nc.tensor.transpose(
                            out=ptr[:, k * 128:(k + 1) * 128].bitcast(F32R),
                            in_=cur[k][:, dc * 128:(dc + 1) * 128],
                            identity=ident_sb,
                        )
                    dst = s["xt"][dc][:, tg * 512:(tg + 1) * 512]
                    if dc % 2 == 0:
                        nc.vector.tensor_copy(dst, ptr)
                    else:
                        nc.scalar.copy(dst, ptr)
                    yield
        return gen()

    def emit_b(idx):
        # projections + RoPE + v
        b, h = pairs[idx]
        s = st[idx]
        xt = s["xt"]
        wqk, wqkr, wv = s["wqk"], s["wqkr"], s["wv"]
        qkr = qkp.tile([128, T], F32R, tag="qkr", name="qkr")
        kq = krsp.tile([128, T], F32R, tag="kq", name="kq")
        s["qkr"], s["kq"] = qkr, kq
        for ncx in range(NC4):
            tsl = slice(ncx * 512, (ncx + 1) * 512)
            pq = psum.tile([128, 512], F32, tag="proj", name="pq")
            for dc in range(ND):
                nc.tensor.matmul(pq, lhsT=wqk[:, dc, :], rhs=xt[dc][:, tsl],
                                 start=(dc == 0), stop=(dc == ND - 1))
            qkc = tmpp.tile([128, 512], F32, tag="qkc", name="qkc")
            nc.vector.tensor_mul(qkc, pq, s["cos"][:, tsl])
            pr = psum.tile([128, 512], F32, tag="proj", name="pr")
            for dc in range(ND):
                nc.tensor.matmul(pr, lhsT=wqkr[:, dc, :], rhs=xt[dc][:, tsl],
                                 start=(dc == 0), stop=(dc == ND - 1))
            qks = tmpp.tile([128, 512], F32, tag="qks", name="qks")
            nc.vector.tensor_mul(qks, pr, s["sin"][:, tsl])
            nc.vector.tensor_add(qkr[:, tsl], qkc, qks)
            # re-align across partitions: k down to 0-63, q up to 64-127
            nc.sync.dma_start(out=kq[0:64, tsl], in_=qkr[64:128, tsl])
            nc.sync.dma_start(out=kq[64:128, tsl], in_=qkr[0:64, tsl])
        # v^T pass then transpose to natural [s, 64] (+ ones column)
        vt = vtp.tile([64, T], F32, tag="vt", name="vt")
        for ncx in range(NC4):
            tsl = slice(ncx * 512, (ncx + 1) * 512)
            pv = psum.tile([128, 512], F32, tag="proj", name="pv")
            pv64 = pv[:64, :]
            for dc in range(ND):
                nc.tensor.matmul(pv64, lhsT=wv[:, dc, :], rhs=xt[dc][:, tsl],
                                 start=(dc == 0), stop=(dc == ND - 1))
            nc.vector.tensor_copy(vt[:, tsl], pv64)
        vn = vnp.tile([128, NT, HEAD_DIM + 2], F32R, tag="vn", name="vn")
        s["vn"] = vn
        vones_bcast = bass.AP(
            tensor=vones_sb.tensor,
            offset=vones_sb.offset,
            ap=[vones_sb.ap[0], [0, NT], vones_sb.ap[1]],
        )
        nc.sync.dma_start(out=vn[:, :, HEAD_DIM:HEAD_DIM + 2], in_=vones_bcast)
        for si in range(NT):
            pvt = psum.tile([128, 512], F32, tag="proj", name="pvt")
            nc.tensor.transpose(
                out=pvt[:, 0:HEAD_DIM],
                in_=vt[:, si * 128:(si + 1) * 128],
                identity=ident_sb[0:64, 0:64].bitcast(F32),
            )
            nc.vector.tensor_copy(vn[:, si, 0:HEAD_DIM], pvt[:, 0:HEAD_DIM])

    def att_mm(patt, vn, si, pt, n_s):
        nc.tensor.matmul(patt, lhsT=vn[:, si, :], rhs=pt,
                         start=(si == 0), stop=(si == n_s - 1),
                         skip_group_check=True)

    def make_finisher(idx, tcx, att_sb):
        b, h = pairs[idx]
        s = st[idx]
        wo = s["wo"]

        def fin():
            pdn = psum_dn.tile([128, 4], F32, tag="dn", name="pdn")
            for k in range(4):
                nc.tensor.transpose(
                    out=pdn[:, k:k + 1],
                    in_=att_sb[HEAD_DIM:HEAD_DIM + 1,
                               k * 128:(k + 1) * 128].bitcast(F32),
                    identity=ones_sb[HEAD_DIM:HEAD_DIM + 1, :],
                )
            # duplicate att rows into partitions 64-127 for row-packed o_proj
            nc.sync.dma_start(out=att_sb[HEAD_DIM:2 * HEAD_DIM, :],
                              in_=att_sb[0:HEAD_DIM, :])
            ra = rap.tile([128, 4], F32, tag="ra", name="ra")
            nc.vector.tensor_scalar_add(ra, pdn, DENOM_EPS)
            nc.vector.reciprocal(ra, ra)
            nc.vector.tensor_mul(ra, ra, s["act01"][:, tcx * 4:tcx * 4 + 4])
            for k in (0, 2, 1, 3):
                ti = tcx * 4 + k
                osb = outp.tile([128, HIDDEN], F32, tag="osb", name="osb")
                for dh in range(2):
                    po = psum_o.tile([128, 512], F32, tag="o", name="po")
                    if k % 2 == 0:
                        nc.tensor.matmul(
                            po,
                            lhsT=att_sb[0:HEAD_DIM, k * 128:(k + 1) * 128],
                            rhs=wo[0:HEAD_DIM, dh * 512:(dh + 1) * 512],
                            start=True, stop=True,
                        )
                    else:
                        nc.tensor.matmul(
                            po,
                            lhsT=att_sb[HEAD_DIM:2 * HEAD_DIM,
                                        k * 128:(k + 1) * 128],
                            rhs=wo[HEAD_DIM:2 * HEAD_DIM,
                                   dh * 512:(dh + 1) * 512],
                            start=True, stop=True,
                        )
                    dst = osb[:, dh * 512:(dh + 1) * 512]
                    if (k + dh) % 2 == 0:
                        nc.vector.tensor_scalar_mul(dst, po, ra[:, k:k + 1])
                    else:
                        nc.scalar.mul(dst, po, ra[:, k:k + 1])
                nc.sync.dma_start(
                    out=out_d[b, h, ti * 128:(ti + 1) * 128, :], in_=osb)
        return fin

    def emit_c_chunk(idx, tcx, filler=None):
        s = st[idx]
        qkr, kq, vn = s["qkr"], s["kq"], s["vn"]
        tsl = slice(tcx * 512, (tcx + 1) * 512)
        n_s = 4 * (tcx + 1)
        patt = psum_att.tile([HEAD_DIM + 2, 512], F32, tag="att", name="patt")
        prob_tiles = []
        for si in range(n_s):
            psc = psum_sc.tile([128, 512], F32, tag="sc", name="psc")
            if si % 2 == 0:
                nc.tensor.matmul(
                    psc,
                    lhsT=kq[0:64, si * 128:(si + 1) * 128],
                    rhs=qkr[0:64, tsl],
                    start=True, stop=True,
                )
            else:
                nc.tensor.matmul(
                    psc,
                    lhsT=qkr[64:128, si * 128:(si + 1) * 128],
                    rhs=kq[64:128, tsl],
                    start=True, stop=True,
                )
            pt = probp.tile([128, 512], F32R, tag="prob", name="pt")
            kd = si - 4 * tcx
            nc.scalar.activation(pt, psc, mybir.ActivationFunctionType.Exp,
                                 bias=s["actb"][:, si:si + 1])
            if kd >= 0:
                nc.vector.tensor_mul(pt, pt, dmask_sb[:, kd, :])
            prob_tiles.append(pt)
            if filler is not None:
                next(filler, None)
            if si >= 2:
                att_mm(patt, vn, si - 2, prob_tiles[si - 2], n_s)
        att_mm(patt, vn, n_s - 2, prob_tiles[n_s - 2], n_s)
        att_mm(patt, vn, n_s - 1, prob_tiles[n_s - 1], n_s)
        att_sb = attp.tile([128, 512], F32R, tag="attsb", name="att_sb")
        nc.vector.tensor_copy(att_sb[0:HEAD_DIM + 2, :], patt)
        if pending:
            pending.pop(0)()
        pending.append(make_finisher(idx, tcx, att_sb))

    # ---------- interleaved pipeline across pairs ----------
    emit_tables(0)
    for _ in emit_a_steps(0):
        pass
    for idx in range(n_pairs):
        emit_b(idx)
        filler = None
        for tcx in range(NC4):
            if idx + 1 < n_pairs and tcx == 0:
                emit_tables(idx + 1)
                filler = emit_a_steps(idx + 1)
            emit_c_chunk(idx, tcx, filler)
        if filler is not None:
            for _ in filler:
                pass
        if idx > 0:
            del st[idx - 1]
    while pending:
        pending.pop(0)()



_PROGRAM = None


def kernel(**inputs) -> np.ndarray:
    global _PROGRAM
    (x, cosT, sinT, wqk, wqkr, wv, wo, actb, act01, dmask, vones,
     ident) = _host_prep(inputs)

    if _PROGRAM is None:
        _PROGRAM = _build_program()
    nc = _PROGRAM

    in_maps = []
    for c in range(NCORES):
        hs = slice(c * HPC, (c + 1) * HPC)
        in_maps.append({
            "x": np.ascontiguousarray(x[:, hs]),
            "cos": np.ascontiguousarray(cosT[:, hs]),
            "sin": np.ascontiguousarray(sinT[:, hs]),
            "wqk": np.ascontiguousarray(wqk[hs]),
            "wqkr": np.ascontiguousarray(wqkr[hs]),
            "wv": np.ascontiguousarray(wv[hs]),
            "wo": np.ascontiguousarray(wo[hs]),
            "actb": np.ascontiguousarray(actb[:, hs]),
            "act01": np.ascontiguousarray(act01[:, hs]),
            "dmask": dmask,
            "vones": vones,
            "ident": ident,
        })

    res = run_bass_kernel_spmd(nc, in_maps, list(range(NCORES)))
    outs = [res.results[c]["out"] for c in range(NCORES)]
    return np.concatenate(outs, axis=1).astype(np.float32)



# revision 15
# speedup vs baseline: 2.3619x; 2.3619x over previous
"""Trainium2 Bass kernel for BottleneckedEnsembleAttention (sparse/compacted).

Sharding: 8 cores, core c handles heads [2c, 2c+1] for both batches
(4 independent (b, head) attention problems per core).

Sparsity: the reference zeroes output rows for inactive queries, masks
inactive keys out of the softmax, and inactive tokens never otherwise
contribute.  The host therefore COMPACTS each (b, h) problem to its active
tokens (order-preserving, so the causal mask stays lower-triangular), pads
to NA = ceil(max_active/128)*128, and scatters the device output back into
a zero tensor.  Seed-0 counts are ~1024 of 2048, so NA = 1152: projections
shrink ~2x and attention area ~3.2x.

Host also pre-transposes the compacted X to [HIDDEN, NA] fp16 (no on-device
transposes for X), folds the softmax scale into Wq, and computes compacted
YaRN cos/sin tables [32, NA] fp16 (rows are 32-periodic on device).

Per (b, h) on-device pipeline (all heavy matmuls 1 cycle/col):
  1. qk pass: psum_qk = [Wq*scale | Wk]^T X^T  -> [128, ch] (q^T rows 0-63,
     k^T rows 64-127), fp16 inputs.
  2. RoPE on PE: ev_c = psum_qk * cos, ev_s = psum_qk * sin (DVE);
     cos/sin rows are 32-periodic so rotate-half/q-k-swap permutations
     commute with the elementwise multiplies:
       qkrot = P_rot @ ev_s + ev_c          (2 matmuls into one psum)
       kq    = (Psw P_rot) @ ev_s + Psw @ ev_c   (k in rows 0-63, q in 64-127)
  3. v^T pass (fp16) -> vt [65, NA] with row 64 = active-indicator (for the
     free softmax denominator), PE-transposed to vn [s, 65].
  4. per t-chunk (384 cols): scores^T[s-tile, t] = k lhsT @ q rhs; causal
     mask added INSIDE the matmul via an fp16 strict-upper -60000 triangular
     lhsT against an identity rhs on the diagonal 128-block; exp via ACT;
     att^T[u, t] accumulated over s-tiles (row 64 = denominator);
     o_proj: out[t-tile, 1024] = att^T lhsT @ Wo rhs, scaled by 1/denom at
     PSUM eviction (denom reciprocal as a row, PE-transposed to columns).
  5. store compacted fp16 output rows; host upcasts and scatters.

The emission is software-pipelined across the 4 (b, h) problems: the next
pair's projections/RoPE/v are interleaved as PE gap-filler inside the
current pair's attention chunks, and each chunk's o_proj/store is deferred
one chunk so the PE never waits on eviction chains.  PSUM evictions are
spread across ACT/DVE/Pool engines.
"""

import math
from contextlib import ExitStack

import numpy as np

import concourse.bass as bass
import concourse.mybir as mybir
import concourse.tile as tile
from concourse import bacc
from concourse.bass_utils import run_bass_kernel_spmd

# model constants (must match reference.py)
HIDDEN = 1024
HEADS = 16
HEAD_DIM = 64
THETA = 10000.0
TRAIN_LEN = 2048
SCALE = 4.0
ALPHA = 1.0
BETA = 32.0
B, T = 2, 2048

NCORES = 8
HPC = HEADS // NCORES  # heads per core = 2
NPAIRS = B * HPC       # independent (b, h) problems per core = 4

F32 = mybir.dt.float32
F32R = mybir.dt.float32r
F16 = mybir.dt.float16
BF16 = mybir.dt.bfloat16

ND = HIDDEN // 128  # 8 d-chunks
CW = 384            # chunk width (3 t-tiles)
TPC = CW // 128     # t-tiles per chunk = 3
NEG_TRI = -60000.0  # fp16-representable; exp(score + NEG_TRI) == 0.0


def _yarn_inv_freq():
    half = HEAD_DIM // 2
    pos_freqs = THETA ** (np.arange(half, dtype=np.float32) * 2.0 / HEAD_DIM)
    inv_freq_extra = (1.0 / pos_freqs).astype(np.float32)
    inv_freq_inter = (1.0 / (SCALE * pos_freqs)).astype(np.float32)

    def find_dim(num_rot):
        return (HEAD_DIM * math.log(TRAIN_LEN / (num_rot * 2.0 * math.pi))) / (
            2.0 * math.log(THETA)
        )

    low = max(math.floor(find_dim(BETA)), 0)
    high = min(math.ceil(find_dim(ALPHA)), half - 1)
    ramp = np.clip(
        (np.arange(half, dtype=np.float32) - low) / max(high - low, 1e-3), 0.0, 1.0
    ).astype(np.float32)
    extrap = (1.0 - ramp).astype(np.float32)
    inv_freq = inv_freq_inter * (1.0 - extrap) + inv_freq_extra * extrap
    mscale = 0.1 * math.log(SCALE) + 1.0 if SCALE > 1.0 else 1.0
    return inv_freq.astype(np.float32), np.float32(mscale)


def _perm_consts():
    """Permutation lhsT matrices for RoPE on the PE.

    P_rot: within each 64-row block (q rows 0-63, k rows 64-127),
      (P v)[u] = -v[u+32] for u<32, +v[u-32] for u>=32  (rotate-half w/ sign)
    P_swap: (P v)[u] = v[(u+64) % 128]                  (q<->k block swap)
    Matmul computes lhsT.T @ rhs, so pass the TRANSPOSE of each matrix.
    """
    P_rot = np.zeros((128, 128), dtype=np.float32)
    for blk in (0, 64):
        for u in range(32):
            P_rot[blk + u, blk + u + 32] = -1.0
            P_rot[blk + u + 32, blk + u] = 1.0
    P_swap = np.zeros((128, 128), dtype=np.float32)
    for u in range(128):
        P_swap[u, (u + 64) % 128] = 1.0
    P_swrot = P_swap @ P_rot
    ident = np.eye(128, dtype=np.float32)
    # [4, 128, 128]: lhsT variants (transposed), identity last
    perms = np.stack(
        [P_rot.T, P_swrot.T, P_swap.T, ident], axis=0
    )
    return np.ascontiguousarray(perms)


def _tri_consts():
    """fp16 [3, 128, 128]: slot 0 = M^T where M[s,t] = NEG_TRI for s > t
    (strict lower triangle in (s, t)), slot 1 = identity, slot 2 = all
    NEG_TRI (for fully-masked s>t blocks left of the diagonal)."""
    M = np.tril(np.full((128, 128), NEG_TRI, dtype=np.float32), k=-1)
    full = np.full((128, 128), NEG_TRI, dtype=np.float32)
    out = np.stack([M.T, np.eye(128, dtype=np.float32), full], axis=0)
    return np.ascontiguousarray(out.astype(np.float16))


def _host_prep(inputs):
    x = np.asarray(inputs["packed_embeddings"], dtype=np.float32)
    pos = np.asarray(inputs["position_ids"])
    act = np.asarray(inputs["active_mask"])
    wq = np.asarray(inputs["q_proj"], dtype=np.float32)
    wk = np.asarray(inputs["k_proj"], dtype=np.float32)
    wv = np.asarray(inputs["v_proj"], dtype=np.float32)
    wo = np.asarray(inputs["o_proj"], dtype=np.float32)

    inv_freq, mscale = _yarn_inv_freq()
    scale = np.float32(mscale / math.sqrt(HEAD_DIM))

    counts = act.sum(axis=-1)  # (B, HEADS)
    nt_act = max(1, int(-(-counts.max() // 128)))
    na = nt_act * 128
    # round tiles up to a multiple of TPC so chunks are uniform
    nt_act = -(-nt_act // TPC) * TPC
    na = nt_act * 128

    idx = [[np.nonzero(act[b, l])[0] for l in range(HEADS)] for b in range(B)]

    xt = np.zeros((B, HEADS, HIDDEN, na), dtype=np.float16)
    cs = np.zeros((B, HEADS, 2, HEAD_DIM // 2, na), dtype=np.float16)
    a01 = np.zeros((B, HEADS, 1, na), dtype=np.float32)
    for b in range(B):
        for l in range(HEADS):
            ii = idx[b][l]
            n = len(ii)
            xt[b, l, :, :n] = x[b, l, ii, :].T
            ang = pos[b, l, ii].astype(np.float32)[:, None] * inv_freq  # (n, 32)
            cs[b, l, 0, :, :n] = np.cos(ang).T
            cs[b, l, 1, :, :n] = np.sin(ang).T
            a01[b, l, 0, :n] = 1.0

    wqk = np.concatenate([wq * scale, wk], axis=-1)  # (L, 1024, 128)
    wqk16 = np.ascontiguousarray(wqk.astype(np.float16))
    wv16 = np.ascontiguousarray(wv.astype(np.float16))
    wo32 = np.ascontiguousarray(wo)

    perms = _perm_consts()
    tri = _tri_consts()
    meta = {"na": na, "nt_act": nt_act, "counts": counts, "idx": idx}
    return xt, cs, a01, wqk16, wv16, wo32, perms, tri, meta


def _build_program(na):
    nt = na // 128
    nc = bacc.Bacc("TRN2", target_bir_lowering=False, debug=False)

    xt_d = nc.declare_dram_parameter("xt", [B, HPC, HIDDEN, na], F16, isOutput=False)
    cs_d = nc.declare_dram_parameter("cs", [B, HPC, 2, HEAD_DIM // 2, na], F16,
                                     isOutput=False)
    a01_d = nc.declare_dram_parameter("a01", [B, HPC, 1, na], F32, isOutput=False)
    wqk_d = nc.declare_dram_parameter("wqk", [HPC, HIDDEN, 128], F16, isOutput=False)
    wv_d = nc.declare_dram_parameter("wv", [HPC, HIDDEN, HEAD_DIM], F16,
                                     isOutput=False)
    wo_d = nc.declare_dram_parameter("wo", [HPC, HEAD_DIM, HIDDEN], F32R,
                                     isOutput=False)
    perm_d = nc.declare_dram_parameter("perm", [4, 128, 128], F32R, isOutput=False)
    tri_d = nc.declare_dram_parameter("tri", [3, 128, 128], F16, isOutput=False)
    out_d = nc.declare_dram_parameter("out", [B, HPC, na, HIDDEN], F16, isOutput=True)

    with ExitStack() as ctx:
        tc = ctx.enter_context(tile.TileContext(nc))
        _emit(ctx, tc, nc, na, nt, xt_d, cs_d, a01_d, wqk_d, wv_d, wo_d,
              perm_d, tri_d, out_d)
    nc.compile()
    return nc


def _emit(ctx, tc, nc, na, nt, xt_d, cs_d, a01_d, wqk_d, wv_d, wo_d,
          perm_d, tri_d, out_d):
    nchunks = nt // TPC

    # ---- pools ----
    consts = ctx.enter_context(tc.tile_pool(name="consts", bufs=1))
    wpool = ctx.enter_context(tc.tile_pool(name="wpool", bufs=2))
    xtp = ctx.enter_context(tc.tile_pool(name="xt", bufs=2))
    cssp = ctx.enter_context(tc.tile_pool(name="css", bufs=2))
    qkp = ctx.enter_context(tc.tile_pool(name="qk", bufs=2))
    evp = ctx.enter_context(tc.tile_pool(name="ev", bufs=2))
    vtp = ctx.enter_context(tc.tile_pool(name="vt", bufs=2))
    vnp = ctx.enter_context(tc.tile_pool(name="vn", bufs=2))
    ptp = ctx.enter_context(tc.tile_pool(name="pt", bufs=4))
    attp = ctx.enter_context(tc.tile_pool(name="att", bufs=2))
    rap = ctx.enter_context(tc.tile_pool(name="ra", bufs=2))
    outp = ctx.enter_context(tc.tile_pool(name="outsb", bufs=2))

    ps_proj = ctx.enter_context(tc.tile_pool(name="ps_proj", bufs=2, space="PSUM"))
    ps_rk = ctx.enter_context(tc.tile_pool(name="ps_rk", bufs=2, space="PSUM"))
    ps_sc = ctx.enter_context(tc.tile_pool(name="ps_sc", bufs=2, space="PSUM"))
    ps_att = ctx.enter_context(tc.tile_pool(name="ps_att", bufs=1, space="PSUM"))
    ps_o = ctx.enter_context(tc.tile_pool(name="ps_o", bufs=1, space="PSUM"))

    # ---- constants (once) ----
    perm_sb = consts.tile([128, 4, 128], F32R)
    nc.sync.dma_start(out=perm_sb, in_=perm_d.rearrange("k p m -> p k m"))
    tri_sb = consts.tile([128, 3, 128], F16)
    nc.sync.dma_start(out=tri_sb, in_=tri_d.rearrange("k p m -> p k m"))
    ones_sb = consts.tile([128, 1], F32)
    nc.vector.memset(ones_sb, 1.0)
    # [128, 256] fp16 = [I | I] rhs for full-block masking (column sums 1)
    ident_wide = consts.tile([128, (TPC - 1) * 128], F16)
    for j in range(TPC - 1):
        nc.sync.dma_start(out=ident_wide[:, j * 128:(j + 1) * 128],
                          in_=tri_d[1])

    pairs = [(b, h) for b in range(B) for h in range(HPC)]
    st = {}       # per-pair state
    pending = []  # deferred chunk finishers

    # ---------- phase emitters ----------
    def emit_tables(idx):
        b, h = pairs[idx]
        s = st[idx] = {}
        s["cos"] = cssp.tile([128, na], F16, tag="cos", name="cos_sb")
        base = cs_d[b, h, 0]
        nc.sync.dma_start(out=s["cos"], in_=bass.AP(
            tensor=base.tensor, offset=base.offset, ap=[[0, 4]] + list(base.ap)))
        s["sin"] = cssp.tile([128, na], F16, tag="sin", name="sin_sb")
        base = cs_d[b, h, 1]
        nc.sync.dma_start(out=s["sin"], in_=bass.AP(
            tensor=base.tensor, offset=base.offset, ap=[[0, 4]] + list(base.ap)))
        t_qk = wpool.tile([128, ND, 128], F16, tag="wqk", name="t_qk")
        nc.sync.dma_start(out=t_qk, in_=wqk_d[h].rearrange("(c p) m -> p c m", p=128))
        t_v = wpool.tile([128, ND, HEAD_DIM], F16, tag="wv", name="t_v")
        nc.sync.dma_start(out=t_v, in_=wv_d[h].rearrange("(c p) m -> p c m", p=128))
        t_o = wpool.tile([HEAD_DIM, HIDDEN], F32R, tag="wo", name="t_o")
        nc.sync.dma_start(out=t_o, in_=wo_d[h])
        s["wqk"], s["wv"], s["wo"] = t_qk, t_v, t_o
        # X^T [128, ND, na] fp16, two half-loads for earlier compute start
        s["xt"] = xtp.tile([128, ND, na], F16, tag="xt", name="xt_sb")
        half = ND // 2
        src = xt_d[b, h].rearrange("(c p) t -> p c t", p=128)
        nc.sync.dma_start(out=s["xt"][:, 0:half, :], in_=src[:, 0:half, :])
        nc.sync.dma_start(out=s["xt"][:, half:ND, :], in_=src[:, half:ND, :])
        # vt with active-indicator row 64 (free softmax denominator)
        s["vt"] = vtp.tile([HEAD_DIM + 1, na], F32, tag="vt", name="vt_sb")
        nc.sync.dma_start(out=s["vt"][HEAD_DIM:HEAD_DIM + 1, :],
                          in_=a01_d[b, h])

    def emit_b_steps(idx):
        # projections + RoPE + v for pair idx; generator yields between steps
        s = st[idx]
        xt, wqk, wv = s["xt"], s["wqk"], s["wv"]
        cos, sin = s["cos"], s["sin"]
        qkrot = qkp.tile([128, na], F32R, tag="qkrot", name="qkrot")
        kq = qkp.tile([128, na], F32R, tag="kq", name="kq")
        s["qkrot"], s["kq"] = qkrot, kq
        vt = s["vt"]

        for cx in range(nchunks):
            tsl = slice(cx * CW, (cx + 1) * CW)
            pq = ps_proj.tile([128, CW], F32, tag="proj", name="pq")
            for dc in range(ND):
                nc.tensor.matmul(pq, lhsT=wqk[:, dc, :], rhs=xt[:, dc, tsl],
                                 start=(dc == 0), stop=(dc == ND - 1))
            yield
            ev_c = evp.tile([128, CW], F32R, tag="evc", name="ev_c")
            nc.vector.tensor_mul(ev_c, pq, cos[:, tsl])
            ev_s = evp.tile([128, CW], F32R, tag="evs", name="ev_s")
            nc.vector.tensor_mul(ev_s, pq, sin[:, tsl])
            yield
            pr = ps_rk.tile([128, CW], F32, tag="rk", name="pr")
            nc.tensor.matmul(pr, lhsT=perm_sb[:, 0, :], rhs=ev_s,
                             start=True, stop=False)
            nc.tensor.matmul(pr, lhsT=perm_sb[:, 3, :], rhs=ev_c,
                             start=False, stop=True, skip_group_check=True)
            nc.scalar.copy(qkrot[:, tsl], pr)
            yield
            pk = ps_rk.tile([128, CW], F32, tag="rk", name="pk")
            nc.tensor.matmul(pk, lhsT=perm_sb[:, 1, :], rhs=ev_s,
                             start=True, stop=False)
            nc.tensor.matmul(pk, lhsT=perm_sb[:, 2, :], rhs=ev_c,
                             start=False, stop=True, skip_group_check=True)
            nc.vector.tensor_copy(kq[:, tsl], pk)
            yield
        # v^T pass
        for cx in range(nchunks):
            tsl = slice(cx * CW, (cx + 1) * CW)
            pv = ps_proj.tile([128, CW], F32, tag="proj", name="pv")
            pv64 = pv[0:HEAD_DIM, :]
            for dc in range(ND):
                nc.tensor.matmul(pv64, lhsT=wv[:, dc, :], rhs=xt[:, dc, tsl],
                                 start=(dc == 0), stop=(dc == ND - 1))
            yield
            nc.vector.tensor_copy(vt[0:HEAD_DIM, tsl], pv64)
            yield
        # v natural [s, 65] via PE transposes, packed into 2 psum tiles
        vcols = HEAD_DIM + 1
        vn = vnp.tile([128, nt * vcols], F32R, tag="vn", name="vn")
        s["vn"] = vn
        groups = [(0, 5), (5, nt)] if nt > 5 else [(0, nt)]
        for g0, g1 in groups:
            pvt = ps_rk.tile([128, CW], F32, tag="rk", name="pvt")
            for si in range(g0, g1):
                nc.tensor.transpose(
                    out=pvt[:, (si - g0) * vcols:(si - g0 + 1) * vcols],
                    in_=vt[:, si * 128:(si + 1) * 128],
                    identity=perm_sb[0:vcols, 3, 0:vcols].bitcast(F32),
                )
            nc.vector.tensor_copy(
                vn[:, g0 * vcols:g1 * vcols],
                pvt[:, 0:(g1 - g0) * vcols])
            yield

    def make_finisher(idx, cx, att_sb, ra):
        b, h = pairs[idx]
        s = st[idx]
        wo = s["wo"]

        def fin():
            for k in range(TPC):
                ti = cx * TPC + k
                osb = outp.tile([128, HIDDEN], F16, tag="osb", name="osb")
                for dh in range(2):
                    po = ps_o.tile([128, 512], F32, tag="o", name="po")
                    nc.tensor.matmul(
                        po,
                        lhsT=att_sb[0:HEAD_DIM, k * 128:(k + 1) * 128],
                        rhs=wo[:, dh * 512:(dh + 1) * 512],
                        start=True, stop=True,
                    )
                    dst = osb[:, dh * 512:(dh + 1) * 512]
                    if (k * 2 + dh) % 2 == 0:
                        nc.scalar.mul(dst, po, ra[:, k:k + 1])
                    else:
                        nc.vector.tensor_scalar_mul(dst, po, ra[:, k:k + 1])
                nc.sync.dma_start(
                    out=out_d[b, h, ti * 128:(ti + 1) * 128, :], in_=osb)
        return fin

    def emit_c_chunk(idx, cx, filler=None):
        s = st[idx]
        qkrot, kq, vn = s["qkrot"], s["kq"], s["vn"]
        tsl = slice(cx * CW, (cx + 1) * CW)
        n_s = TPC * (cx + 1)
        patt = ps_att.tile([HEAD_DIM + 1, CW], F32, tag="att", name="patt")
        prob_tiles = []

        vcols = HEAD_DIM + 1

        def att_mm(si):
            nc.tensor.matmul(patt, lhsT=vn[:, si * vcols:(si + 1) * vcols],
                             rhs=prob_tiles[si],
                             start=(si == 0), stop=(si == n_s - 1),
                             skip_group_check=True)

        for si in range(n_s):
            psc = ps_sc.tile([128, CW], F32, tag="sc", name="psc")
            kd = si - TPC * cx
            diag = kd >= 0
            if si % 2 == 0:
                nc.tensor.matmul(
                    psc,
                    lhsT=kq[0:HEAD_DIM, si * 128:(si + 1) * 128],
                    rhs=qkrot[0:HEAD_DIM, tsl],
                    start=True, stop=not diag,
                )
            else:
                nc.tensor.matmul(
                    psc,
                    lhsT=qkrot[HEAD_DIM:128, si * 128:(si + 1) * 128],
                    rhs=kq[HEAD_DIM:128, tsl],
                    start=True, stop=not diag,
                )
            if diag:
                # columns left of the diagonal block are s > t: fully mask
                if kd > 0:
                    nc.tensor.matmul(
                        psc[:, 0:kd * 128],
                        lhsT=tri_sb[:, 2, :],
                        rhs=ident_wide[:, 0:kd * 128],
                        start=False, stop=False, skip_group_check=True,
                    )
                nc.tensor.matmul(
                    psc[:, kd * 128:(kd + 1) * 128],
                    lhsT=tri_sb[:, 0, :], rhs=tri_sb[:, 1, :],
                    start=False, stop=True, skip_group_check=True,
                )
            pt = ptp.tile([128, CW], F32R, tag="pt", name="pt")
            nc.scalar.activation(pt, psc, mybir.ActivationFunctionType.Exp)
            prob_tiles.append(pt)
            if filler is not None:
                next(filler, None)
            if si >= 2:
                att_mm(si - 2)
        att_mm(n_s - 2)
        att_mm(n_s - 1)

        # denominator reciprocal as a row, then PE-transpose to columns
        rc = rap.tile([1, CW], F32, tag="rc", name="rc")
        nc.vector.reciprocal(rc, patt[HEAD_DIM:HEAD_DIM + 1, :])
        att_sb = attp.tile([HEAD_DIM + 1, CW], F32R, tag="attsb", name="att_sb")
        if cx % 2 == 0:
            nc.scalar.copy(att_sb, patt)
        else:
            nc.vector.tensor_copy(att_sb, patt)
        pra = ps_sc.tile([128, CW], F32, tag="sc", name="pra")
        for k in range(TPC):
            nc.tensor.transpose(
                out=pra[:, k:k + 1],
                in_=rc[0:1, k * 128:(k + 1) * 128],
                identity=ones_sb[0:1, :],
            )
        ra = rap.tile([128, TPC], F32, tag="ra", name="ra")
        nc.vector.tensor_copy(ra, pra[:, 0:TPC])

        if pending:
            pending.pop(0)()
        pending.append(make_finisher(idx, cx, att_sb, ra))

    # ---------- interleaved pipeline across pairs ----------
    emit_tables(0)
    for _ in emit_b_steps(0):
        pass
    for idx in range(NPAIRS):
        filler = None
        if idx + 1 < NPAIRS:
            emit_tables(idx + 1)
            filler = emit_b_steps(idx + 1)
        for cx in range(nchunks):
            emit_c_chunk(idx, cx, filler)
        if filler is not None:
            for _ in filler:
                pass
        if idx > 0:
            del st[idx - 1]
    while pending:
        pending.pop(0)()


_PROGRAM = {}


def _prep_in_maps(inputs):
    xt, cs, a01, wqk16, wv16, wo32, perms, tri, meta = _host_prep(inputs)
    in_maps = []
    for c in range(NCORES):
        hs = slice(c * HPC, (c + 1) * HPC)
        in_maps.append({
            "xt": np.ascontiguousarray(xt[:, hs]),
            "cs": np.ascontiguousarray(cs[:, hs]),
            "a01": np.ascontiguousarray(a01[:, hs]),
            "wqk": np.ascontiguousarray(wqk16[hs]),
            "wv": np.ascontiguousarray(wv16[hs]),
            "wo": np.ascontiguousarray(wo32[hs]),
            "perm": perms,
            "tri": tri,
        })
    return in_maps, meta


def kernel(**inputs) -> np.ndarray:
    in_maps, meta = _prep_in_maps(inputs)
    na = meta["na"]

    if na not in _PROGRAM:
        _PROGRAM[na] = _build_program(na)
    nc = _PROGRAM[na]

    res = run_bass_kernel_spmd(nc, in_maps, list(range(NCORES)))

    out = np.zeros((B, HEADS, T, HIDDEN), dtype=np.float32)
    idx = meta["idx"]
    for c in range(NCORES):
        oc = res.results[c]["out"]  # [B, HPC, na, HIDDEN] fp16
        for b in range(B):
            for hh in range(HPC):
                l = c * HPC + hh
                ii = idx[b][l]
                out[b, l, ii, :] = oc[b, hh, :len(ii), :].astype(np.float32)
    return out


# revision 18
# speedup vs baseline: 2.7639x; 1.1702x over previous
"""Trainium2 Bass kernel for BottleneckedEnsembleAttention (sparse/compacted).

Sharding: 8 cores, core c handles heads [2c, 2c+1] for both batches
(4 independent (b, head) attention problems per core).

Sparsity: the reference zeroes output rows for inactive queries, masks
inactive keys out of the softmax, and inactive tokens never otherwise
contribute.  The host therefore COMPACTS each (b, h) problem to its active
tokens (order-preserving, so the causal mask stays lower-triangular), pads
to NA = ceil(max_active/128)*128, and scatters the device output back into
a zero tensor.  Seed-0 counts are ~1024 of 2048, so NA = 1152: projections
shrink ~2x and attention area ~3.2x.

Host also pre-transposes the compacted X to [HIDDEN, NA] fp16 (no on-device
transposes for X), folds the softmax scale into Wq, and computes compacted
YaRN cos/sin tables [32, NA] fp16 (rows are 32-periodic on device).

Per (b, h) on-device pipeline (all heavy matmuls 1 cycle/col):
  1. qk pass: psum_qk = [Wq*scale | Wk]^T X^T  -> [128, ch] (q^T rows 0-63,
     k^T rows 64-127), fp16 inputs.
  2. RoPE on PE: ev_c = psum_qk * cos, ev_s = psum_qk * sin (DVE);
     cos/sin rows are 32-periodic so rotate-half/q-k-swap permutations
     commute with the elementwise multiplies:
       qkrot = P_rot @ ev_s + ev_c          (2 matmuls into one psum)
       kq    = (Psw P_rot) @ ev_s + Psw @ ev_c   (k in rows 0-63, q in 64-127)
  3. v^T pass (fp16) -> vt [65, NA] with row 64 = active-indicator (for the
     free softmax denominator), PE-transposed to vn [s, 65].
  4. per t-chunk (384 cols): scores^T[s-tile, t] = k lhsT @ q rhs; causal
     mask added INSIDE the matmul via an fp16 strict-upper -60000 triangular
     lhsT against an identity rhs on the diagonal 128-block; exp via ACT;
     att^T[u, t] accumulated over s-tiles (row 64 = denominator);
     o_proj: out[t-tile, 1024] = att^T lhsT @ Wo rhs, scaled by 1/denom at
     PSUM eviction (denom reciprocal as a row, PE-transposed to columns).
  5. store compacted fp16 output rows; host upcasts and scatters.

The emission is software-pipelined across the 4 (b, h) problems: the next
pair's projections/RoPE/v are interleaved as PE gap-filler inside the
current pair's attention chunks, and each chunk's o_proj/store is deferred
one chunk so the PE never waits on eviction chains.  PSUM evictions are
spread across ACT/DVE/Pool engines.
"""

import math
from contextlib import ExitStack

import numpy as np

import concourse.bass as bass
import concourse.mybir as mybir
import concourse.tile as tile
from concourse import bacc
from concourse.bass_utils import run_bass_kernel_spmd

# model constants (must match reference.py)
HIDDEN = 1024
HEADS = 16
HEAD_DIM = 64
THETA = 10000.0
TRAIN_LEN = 2048
SCALE = 4.0
ALPHA = 1.0
BETA = 32.0
B, T = 2, 2048

NCORES = 8
HPC = HEADS // NCORES  # heads per core = 2
NPAIRS = B * HPC       # independent (b, h) problems per core = 4

F32 = mybir.dt.float32
F32R = mybir.dt.float32r
F16 = mybir.dt.float16
BF16 = mybir.dt.bfloat16

ND = HIDDEN // 128  # 8 d-chunks
CW = 384            # chunk width (3 t-tiles)
TPC = CW // 128     # t-tiles per chunk = 3
NEG_TRI = -60000.0  # fp16-representable; exp(score + NEG_TRI) == 0.0


def _yarn_inv_freq():
    half = HEAD_DIM // 2
    pos_freqs = THETA ** (np.arange(half, dtype=np.float32) * 2.0 / HEAD_DIM)
    inv_freq_extra = (1.0 / pos_freqs).astype(np.float32)
    inv_freq_inter = (1.0 / (SCALE * pos_freqs)).astype(np.float32)

    def find_dim(num_rot):
        return (HEAD_DIM * math.log(TRAIN_LEN / (num_rot * 2.0 * math.pi))) / (
            2.0 * math.log(THETA)
        )

    low = max(math.floor(find_dim(BETA)), 0)
    high = min(math.ceil(find_dim(ALPHA)), half - 1)
    ramp = np.clip(
        (np.arange(half, dtype=np.float32) - low) / max(high - low, 1e-3), 0.0, 1.0
    ).astype(np.float32)
    extrap = (1.0 - ramp).astype(np.float32)
    inv_freq = inv_freq_inter * (1.0 - extrap) + inv_freq_extra * extrap
    mscale = 0.1 * math.log(SCALE) + 1.0 if SCALE > 1.0 else 1.0
    return inv_freq.astype(np.float32), np.float32(mscale)


def _perm_consts():
    """Permutation lhsT matrices for RoPE on the PE.

    P_rot: within each 64-row block (q rows 0-63, k rows 64-127),
      (P v)[u] = -v[u+32] for u<32, +v[u-32] for u>=32  (rotate-half w/ sign)
    P_swap: (P v)[u] = v[(u+64) % 128]                  (q<->k block swap)
    Matmul computes lhsT.T @ rhs, so pass the TRANSPOSE of each matrix.
    """
    P_rot = np.zeros((128, 128), dtype=np.float32)
    for blk in (0, 64):
        for u in range(32):
            P_rot[blk + u, blk + u + 32] = -1.0
            P_rot[blk + u + 32, blk + u] = 1.0
    P_swap = np.zeros((128, 128), dtype=np.float32)
    for u in range(128):
        P_swap[u, (u + 64) % 128] = 1.0
    P_swrot = P_swap @ P_rot
    ident = np.eye(128, dtype=np.float32)
    # [4, 128, 128]: lhsT variants (transposed), identity last
    perms = np.stack(
        [P_rot.T, P_swrot.T, P_swap.T, ident], axis=0
    )
    return np.ascontiguousarray(perms)


def _tri_consts():
    """fp16 [3, 128, 128]: slot 0 = M^T where M[s,t] = NEG_TRI for s > t
    (strict lower triangle in (s, t)), slot 1 = identity, slot 2 = all
    NEG_TRI (for fully-masked s>t blocks left of the diagonal)."""
    M = np.tril(np.full((128, 128), NEG_TRI, dtype=np.float32), k=-1)
    full = np.full((128, 128), NEG_TRI, dtype=np.float32)
    out = np.stack([M.T, np.eye(128, dtype=np.float32), full], axis=0)
    return np.ascontiguousarray(out.astype(np.float16))


def _host_prep(inputs):
    x = np.asarray(inputs["packed_embeddings"], dtype=np.float32)
    pos = np.asarray(inputs["position_ids"])
    act = np.asarray(inputs["active_mask"])
    wq = np.asarray(inputs["q_proj"], dtype=np.float32)
    wk = np.asarray(inputs["k_proj"], dtype=np.float32)
    wv = np.asarray(inputs["v_proj"], dtype=np.float32)
    wo = np.asarray(inputs["o_proj"], dtype=np.float32)

    inv_freq, mscale = _yarn_inv_freq()
    scale = np.float32(mscale / math.sqrt(HEAD_DIM))

    counts = act.sum(axis=-1)  # (B, HEADS)
    nt_act = max(1, int(-(-counts.max() // 128)))
    na = nt_act * 128
    # round tiles up to a multiple of TPC so chunks are uniform
    nt_act = -(-nt_act // TPC) * TPC
    na = nt_act * 128

    idx = [[np.nonzero(act[b, l])[0] for l in range(HEADS)] for b in range(B)]

    xt = np.zeros((B, HEADS, HIDDEN, na), dtype=np.float16)
    cs = np.zeros((B, HEADS, 2, HEAD_DIM // 2, na), dtype=np.float16)
    a01 = np.zeros((B, HEADS, 1, na), dtype=np.float32)
    for b in range(B):
        for l in range(HEADS):
            ii = idx[b][l]
            n = len(ii)
            xt[b, l, :, :n] = x[b, l, ii, :].T
            ang = pos[b, l, ii].astype(np.float32)[:, None] * inv_freq  # (n, 32)
            cs[b, l, 0, :, :n] = np.cos(ang).T
            cs[b, l, 1, :, :n] = np.sin(ang).T
            a01[b, l, 0, :n] = 1.0

    wqk = np.concatenate([wq * scale, wk], axis=-1)  # (L, 1024, 128)
    wqk16 = np.ascontiguousarray(wqk.astype(np.float16))
    wv16 = np.ascontiguousarray(wv.astype(np.float16))
    wo32 = np.ascontiguousarray(wo)

    perms = _perm_consts()
    tri = _tri_consts()
    meta = {"na": na, "nt_act": nt_act, "counts": counts, "idx": idx}
    return xt, cs, a01, wqk16, wv16, wo32, perms, tri, meta


def _build_program(na):
    nt = na // 128
    nc = bacc.Bacc("TRN2", target_bir_lowering=False, debug=False)

    xt_d = nc.declare_dram_parameter("xt", [B, HPC, HIDDEN, na], F16, isOutput=False)
    cs_d = nc.declare_dram_parameter("cs", [B, HPC, 2, HEAD_DIM // 2, na], F16,
                                     isOutput=False)
    a01_d = nc.declare_dram_parameter("a01", [B, HPC, 1, na], F32, isOutput=False)
    wqk_d = nc.declare_dram_parameter("wqk", [HPC, HIDDEN, 128], F16, isOutput=False)
    wv_d = nc.declare_dram_parameter("wv", [HPC, HIDDEN, HEAD_DIM], F16,
                                     isOutput=False)
    wo_d = nc.declare_dram_parameter("wo", [HPC, HEAD_DIM, HIDDEN], F32R,
                                     isOutput=False)
    perm_d = nc.declare_dram_parameter("perm", [4, 128, 128], F32R, isOutput=False)
    tri_d = nc.declare_dram_parameter("tri", [3, 128, 128], F16, isOutput=False)
    out_d = nc.declare_dram_parameter("out", [B, HPC, na, HIDDEN], F16, isOutput=True)

    with ExitStack() as ctx:
        tc = ctx.enter_context(tile.TileContext(nc))
        _emit(ctx, tc, nc, na, nt, xt_d, cs_d, a01_d, wqk_d, wv_d, wo_d,
              perm_d, tri_d, out_d)
    nc.compile()
    return nc


def _emit(ctx, tc, nc, na, nt, xt_d, cs_d, a01_d, wqk_d, wv_d, wo_d,
          perm_d, tri_d, out_d):
    nchunks = nt // TPC

    # ---- pools ----
    consts = ctx.enter_context(tc.tile_pool(name="consts", bufs=1))
    wpool = ctx.enter_context(tc.tile_pool(name="wpool", bufs=2))
    xtp = ctx.enter_context(tc.tile_pool(name="xt", bufs=2))
    cssp = ctx.enter_context(tc.tile_pool(name="css", bufs=2))
    qkp = ctx.enter_context(tc.tile_pool(name="qk", bufs=2))
    evp = ctx.enter_context(tc.tile_pool(name="ev", bufs=2))
    vtp = ctx.enter_context(tc.tile_pool(name="vt", bufs=2))
    vnp = ctx.enter_context(tc.tile_pool(name="vn", bufs=2))
    ptp = ctx.enter_context(tc.tile_pool(name="pt", bufs=4))
    attp = ctx.enter_context(tc.tile_pool(name="att", bufs=2))
    rap = ctx.enter_context(tc.tile_pool(name="ra", bufs=2))
    outp = ctx.enter_context(tc.tile_pool(name="outsb", bufs=2))

    ps_proj = ctx.enter_context(tc.tile_pool(name="ps_proj", bufs=2, space="PSUM"))
    ps_rk = ctx.enter_context(tc.tile_pool(name="ps_rk", bufs=1, space="PSUM"))
    ps_sc = ctx.enter_context(tc.tile_pool(name="ps_sc", bufs=2, space="PSUM"))
    ps_att = ctx.enter_context(tc.tile_pool(name="ps_att", bufs=1, space="PSUM"))
    ps_o = ctx.enter_context(tc.tile_pool(name="ps_o", bufs=2, space="PSUM"))

    # ---- constants (once) ----
    perm_sb = consts.tile([128, 4, 128], F32R)
    nc.sync.dma_start(out=perm_sb, in_=perm_d.rearrange("k p m -> p k m"))
    tri_sb = consts.tile([128, 3, 128], F16)
    nc.sync.dma_start(out=tri_sb, in_=tri_d.rearrange("k p m -> p k m"))
    ones_sb = consts.tile([128, 1], F32)
    nc.vector.memset(ones_sb, 1.0)

    pairs = [(b, h) for b in range(B) for h in range(HPC)]
    st = {}       # per-pair state
    pending = []  # deferred chunk finishers

    # ---------- phase emitters ----------
    def emit_tables(idx):
        b, h = pairs[idx]
        s = st[idx] = {}
        s["cos"] = cssp.tile([128, na], F16, tag="cos", name="cos_sb")
        base = cs_d[b, h, 0]
        nc.sync.dma_start(out=s["cos"], in_=bass.AP(
            tensor=base.tensor, offset=base.offset, ap=[[0, 4]] + list(base.ap)))
        s["sin"] = cssp.tile([128, na], F16, tag="sin", name="sin_sb")
        base = cs_d[b, h, 1]
        nc.sync.dma_start(out=s["sin"], in_=bass.AP(
            tensor=base.tensor, offset=base.offset, ap=[[0, 4]] + list(base.ap)))
        t_qk = wpool.tile([128, ND, 128], F16, tag="wqk", name="t_qk")
        nc.sync.dma_start(out=t_qk, in_=wqk_d[h].rearrange("(c p) m -> p c m", p=128))
        t_v = wpool.tile([128, ND, HEAD_DIM], F16, tag="wv", name="t_v")
        nc.sync.dma_start(out=t_v, in_=wv_d[h].rearrange("(c p) m -> p c m", p=128))
        t_o = wpool.tile([HEAD_DIM, HIDDEN], F32R, tag="wo", name="t_o")
        nc.sync.dma_start(out=t_o, in_=wo_d[h])
        s["wqk"], s["wv"], s["wo"] = t_qk, t_v, t_o
        # X^T [128, ND, na] fp16, two half-loads for earlier compute start
        s["xt"] = xtp.tile([128, ND, na], F16, tag="xt", name="xt_sb")
        half = ND // 2
        src = xt_d[b, h].rearrange("(c p) t -> p c t", p=128)
        nc.sync.dma_start(out=s["xt"][:, 0:half, :], in_=src[:, 0:half, :])
        nc.sync.dma_start(out=s["xt"][:, half:ND, :], in_=src[:, half:ND, :])
        # vt with active-indicator row 64 (free softmax denominator)
        s["vt"] = vtp.tile([HEAD_DIM + 1, na], F32, tag="vt", name="vt_sb")
        nc.sync.dma_start(out=s["vt"][HEAD_DIM:HEAD_DIM + 1, :],
                          in_=a01_d[b, h])

    def emit_b_steps(idx):
        # projections + RoPE + v for pair idx; generator yields between steps
        s = st[idx]
        xt, wqk, wv = s["xt"], s["wqk"], s["wv"]
        cos, sin = s["cos"], s["sin"]
        qkrot = qkp.tile([128, na], F32R, tag="qkrot", name="qkrot")
        kq = qkp.tile([128, na], F32R, tag="kq", name="kq")
        s["qkrot"], s["kq"] = qkrot, kq
        vt = s["vt"]

        for cx in range(nchunks):
            tsl = slice(cx * CW, (cx + 1) * CW)
            pq = ps_proj.tile([128, CW], F32, tag="proj", name="pq")
            for dc in range(ND):
                nc.tensor.matmul(pq, lhsT=wqk[:, dc, :], rhs=xt[:, dc, tsl],
                                 start=(dc == 0), stop=(dc == ND - 1))
            yield
            ev_c = evp.tile([128, CW], F32R, tag="evc", name="ev_c")
            nc.vector.tensor_mul(ev_c, pq, cos[:, tsl])
            ev_s = evp.tile([128, CW], F32R, tag="evs", name="ev_s")
            nc.vector.tensor_mul(ev_s, pq, sin[:, tsl])
            yield
            pr = ps_rk.tile([128, CW], F32, tag="rk", name="pr")
            nc.tensor.matmul(pr, lhsT=perm_sb[:, 0, :], rhs=ev_s,
                             start=True, stop=False)
            nc.tensor.matmul(pr, lhsT=perm_sb[:, 3, :], rhs=ev_c,
                             start=False, stop=True, skip_group_check=True)
            nc.scalar.copy(qkrot[:, tsl], pr)
            yield
            pk = ps_rk.tile([128, CW], F32, tag="rk", name="pk")
            nc.tensor.matmul(pk, lhsT=perm_sb[:, 1, :], rhs=ev_s,
                             start=True, stop=False)
            nc.tensor.matmul(pk, lhsT=perm_sb[:, 2, :], rhs=ev_c,
                             start=False, stop=True, skip_group_check=True)
            nc.vector.tensor_copy(kq[:, tsl], pk)
            yield
        # v^T pass
        for cx in range(nchunks):
            tsl = slice(cx * CW, (cx + 1) * CW)
            pv = ps_proj.tile([128, CW], F32, tag="proj", name="pv")
            pv64 = pv[0:HEAD_DIM, :]
            for dc in range(ND):
                nc.tensor.matmul(pv64, lhsT=wv[:, dc, :], rhs=xt[:, dc, tsl],
                                 start=(dc == 0), stop=(dc == ND - 1))
            yield
            nc.vector.tensor_copy(vt[0:HEAD_DIM, tsl], pv64)
            yield
        # v natural [s, 65] via PE transposes, packed into 2 psum tiles
        vcols = HEAD_DIM + 1
        vn = vnp.tile([128, nt * vcols], BF16, tag="vn", name="vn")
        s["vn"] = vn
        groups = [(0, 5), (5, nt)] if nt > 5 else [(0, nt)]
        for g0, g1 in groups:
            pvt = ps_rk.tile([128, CW], F32, tag="rk", name="pvt")
            for si in range(g0, g1):
                nc.tensor.transpose(
                    out=pvt[:, (si - g0) * vcols:(si - g0 + 1) * vcols],
                    in_=vt[:, si * 128:(si + 1) * 128],
                    identity=perm_sb[0:vcols, 3, 0:vcols].bitcast(F32),
                )
            nc.vector.tensor_copy(
                vn[:, g0 * vcols:g1 * vcols],
                pvt[:, 0:(g1 - g0) * vcols])
            yield

    def make_finisher(idx, cx, att_sb, ra):
        b, h = pairs[idx]
        s = st[idx]
        wo = s["wo"]

        def fin():
            for k in range(TPC):
                ti = cx * TPC + k
                osb = outp.tile([128, HIDDEN], F16, tag="osb", name="osb")
                for dh in range(2):
                    po = ps_o.tile([128, 512], F32, tag="o", name="po")
                    nc.tensor.matmul(
                        po,
                        lhsT=att_sb[0:HEAD_DIM, k * 128:(k + 1) * 128],
                        rhs=wo[:, dh * 512:(dh + 1) * 512],
                        start=True, stop=True,
                    )
                    dst = osb[:, dh * 512:(dh + 1) * 512]
                    if (k * 2 + dh) % 2 == 0:
                        nc.scalar.mul(dst, po, ra[:, k:k + 1])
                    else:
                        nc.vector.tensor_scalar_mul(dst, po, ra[:, k:k + 1])
                    yield
                nc.sync.dma_start(
                    out=out_d[b, h, ti * 128:(ti + 1) * 128, :], in_=osb)
        return fin()

    def step_pending():
        if pending:
            if next(pending[0], StopIteration) is StopIteration:
                pending.pop(0)

    def drain_oldest():
        if pending:
            gen = pending.pop(0)
            for _ in gen:
                pass

    def emit_c_chunk(idx, cx, filler=None):
        s = st[idx]
        qkrot, kq, vn = s["qkrot"], s["kq"], s["vn"]
        tsl = slice(cx * CW, (cx + 1) * CW)
        n_s = TPC * (cx + 1)
        while len(pending) > 1:
            drain_oldest()
        patt = ps_att.tile([HEAD_DIM + 1, CW], F32, tag="att", name="patt")
        prob_tiles = []
        vcols = HEAD_DIM + 1

        def att_mm(si):
            nc.tensor.matmul(patt, lhsT=vn[:, si * vcols:(si + 1) * vcols],
                             rhs=prob_tiles[si],
                             start=(si == 0), stop=(si == n_s - 1),
                             skip_group_check=True)

        for si in range(n_s):
            psc = ps_sc.tile([128, CW], F32, tag="sc", name="psc")
            kd = si - TPC * cx
            diag = kd >= 0
            if si % 2 == 0:
                nc.tensor.matmul(
                    psc,
                    lhsT=kq[0:HEAD_DIM, si * 128:(si + 1) * 128],
                    rhs=qkrot[0:HEAD_DIM, tsl],
                    start=True, stop=not diag,
                )
            else:
                nc.tensor.matmul(
                    psc,
                    lhsT=qkrot[HEAD_DIM:128, si * 128:(si + 1) * 128],
                    rhs=kq[HEAD_DIM:128, tsl],
                    start=True, stop=not diag,
                )
            if diag:
                nc.tensor.matmul(
                    psc[:, kd * 128:(kd + 1) * 128],
                    lhsT=tri_sb[:, 0, :], rhs=tri_sb[:, 1, :],
                    start=False, stop=True, skip_group_check=True,
                )
            pt = ptp.tile([128, CW], BF16, tag="pt", name="pt")
            if diag and kd > 0:
                # columns left of the diagonal block are fully masked (s > t):
                # zero them on the (otherwise idle) Pool engine and exp the rest
                nc.gpsimd.memset(pt[:, 0:kd * 128], 0.0)
                nc.scalar.activation(pt[:, kd * 128:], psc[:, kd * 128:],
                                     mybir.ActivationFunctionType.Exp)
            else:
                nc.scalar.activation(pt, psc, mybir.ActivationFunctionType.Exp)
            prob_tiles.append(pt)
            if filler is not None:
                next(filler, None)
            step_pending()
            if si >= 2:
                att_mm(si - 2)
        att_mm(n_s - 2)
        att_mm(n_s - 1)

        att_sb = attp.tile([HEAD_DIM + 1, CW], F32R, tag="attsb", name="att_sb")
        if cx % 2 == 0:
            nc.scalar.copy(att_sb, patt)
        else:
            nc.vector.tensor_copy(att_sb, patt)
        # denominator: PE-transpose row 64 to columns, reciprocal (tiny ops)
        pdn = ps_sc.tile([128, CW], F32, tag="sc", name="pdn")
        for k in range(TPC):
            nc.tensor.transpose(
                out=pdn[:, k:k + 1],
                in_=att_sb[HEAD_DIM:HEAD_DIM + 1,
                           k * 128:(k + 1) * 128].bitcast(F32),
                identity=ones_sb[HEAD_DIM:HEAD_DIM + 1, :],
            )
        ra = rap.tile([128, TPC], F32, tag="ra", name="ra")
        nc.vector.tensor_copy(ra, pdn[:, 0:TPC])
        nc.vector.reciprocal(ra, ra)

        pending.append(make_finisher(idx, cx, att_sb, ra))

    # ---------- interleaved pipeline across pairs ----------
    emit_tables(0)
    for _ in emit_b_steps(0):
        pass
    for idx in range(NPAIRS):
        filler = None
        if idx + 1 < NPAIRS:
            emit_tables(idx + 1)
            filler = emit_b_steps(idx + 1)
        for cx in range(nchunks):
            emit_c_chunk(idx, cx, filler)
        if filler is not None:
            for _ in filler:
                pass
        if idx > 0:
            del st[idx - 1]
    while pending:
        drain_oldest()


_PROGRAM = {}


def _prep_in_maps(inputs):
    xt, cs, a01, wqk16, wv16, wo32, perms, tri, meta = _host_prep(inputs)
    in_maps = []
    for c in range(NCORES):
        hs = slice(c * HPC, (c + 1) * HPC)
        in_maps.append({
            "xt": np.ascontiguousarray(xt[:, hs]),
            "cs": np.ascontiguousarray(cs[:, hs]),
            "a01": np.ascontiguousarray(a01[:, hs]),
            "wqk": np.ascontiguousarray(wqk16[hs]),
            "wv": np.ascontiguousarray(wv16[hs]),
            "wo": np.ascontiguousarray(wo32[hs]),
            "perm": perms,
            "tri": tri,
        })
    return in_maps, meta


def kernel(**inputs) -> np.ndarray:
    in_maps, meta = _prep_in_maps(inputs)
    na = meta["na"]

    if na not in _PROGRAM:
        _PROGRAM[na] = _build_program(na)
    nc = _PROGRAM[na]

    res = run_bass_kernel_spmd(nc, in_maps, list(range(NCORES)))

    out = np.zeros((B, HEADS, T, HIDDEN), dtype=np.float32)
    idx = meta["idx"]
    for c in range(NCORES):
        oc = res.results[c]["out"]  # [B, HPC, na, HIDDEN] fp16
        for b in range(B):
            for hh in range(HPC):
                l = c * HPC + hh
                ii = idx[b][l]
                out[b, l, ii, :] = oc[b, hh, :len(ii), :].astype(np.float32)
    return out


# revision 19
# speedup vs baseline: 2.7682x; 1.0016x over previous
"""Trainium2 Bass kernel for BottleneckedEnsembleAttention (sparse/compacted).

Sharding: 8 cores, core c handles heads [2c, 2c+1] for both batches
(4 independent (b, head) attention problems per core).

Sparsity: the reference zeroes output rows for inactive queries, masks
inactive keys out of the softmax, and inactive tokens never otherwise
contribute.  The host therefore COMPACTS each (b, h) problem to its active
tokens (order-preserving, so the causal mask stays lower-triangular), pads
to NA = ceil(max_active/128)*128, and scatters the device output back into
a zero tensor.  Seed-0 counts are ~1024 of 2048, so NA = 1152: projections
shrink ~2x and attention area ~3.2x.

Host also pre-transposes the compacted X to [HIDDEN, NA] fp16 (no on-device
transposes for X), folds the softmax scale into Wq, and computes compacted
YaRN cos/sin tables [32, NA] fp16 (rows are 32-periodic on device).

Per (b, h) on-device pipeline (all heavy matmuls 1 cycle/col):
  1. qk pass: psum_qk = [Wq*scale | Wk]^T X^T  -> [128, ch] (q^T rows 0-63,
     k^T rows 64-127), fp16 inputs.
  2. RoPE on PE: ev_c = psum_qk * cos, ev_s = psum_qk * sin (DVE);
     cos/sin rows are 32-periodic so rotate-half/q-k-swap permutations
     commute with the elementwise multiplies:
       qkrot = P_rot @ ev_s + ev_c          (2 matmuls into one psum)
       kq    = (Psw P_rot) @ ev_s + Psw @ ev_c   (k in rows 0-63, q in 64-127)
  3. v^T pass (fp16) -> vt [65, NA] with row 64 = active-indicator (for the
     free softmax denominator), PE-transposed to vn [s, 65].
  4. per t-chunk (384 cols): scores^T[s-tile, t] = k lhsT @ q rhs; causal
     mask added INSIDE the matmul via an fp16 strict-upper -60000 triangular
     lhsT against an identity rhs on the diagonal 128-block; exp via ACT;
     att^T[u, t] accumulated over s-tiles (row 64 = denominator);
     o_proj: out[t-tile, 1024] = att^T lhsT @ Wo rhs, scaled by 1/denom at
     PSUM eviction (denom reciprocal as a row, PE-transposed to columns).
  5. store compacted fp16 output rows; host upcasts and scatters.

The emission is software-pipelined across the 4 (b, h) problems: the next
pair's projections/RoPE/v are interleaved as PE gap-filler inside the
current pair's attention chunks, and each chunk's o_proj/store is deferred
one chunk so the PE never waits on eviction chains.  PSUM evictions are
spread across ACT/DVE/Pool engines.
"""

import math
from contextlib import ExitStack

import numpy as np

import concourse.bass as bass
import concourse.mybir as mybir
import concourse.tile as tile
from concourse import bacc
from concourse.bass_utils import run_bass_kernel_spmd

# model constants (must match reference.py)
HIDDEN = 1024
HEADS = 16
HEAD_DIM = 64
THETA = 10000.0
TRAIN_LEN = 2048
SCALE = 4.0
ALPHA = 1.0
BETA = 32.0
B, T = 2, 2048

NCORES = 8
HPC = HEADS // NCORES  # heads per core = 2
NPAIRS = B * HPC       # independent (b, h) problems per core = 4

F32 = mybir.dt.float32
F32R = mybir.dt.float32r
F16 = mybir.dt.float16
BF16 = mybir.dt.bfloat16

ND = HIDDEN // 128  # 8 d-chunks
CW = 384            # chunk width (3 t-tiles)
TPC = CW // 128     # t-tiles per chunk = 3
NEG_TRI = -60000.0  # fp16-representable; exp(score + NEG_TRI) == 0.0


def _yarn_inv_freq():
    half = HEAD_DIM // 2
    pos_freqs = THETA ** (np.arange(half, dtype=np.float32) * 2.0 / HEAD_DIM)
    inv_freq_extra = (1.0 / pos_freqs).astype(np.float32)
    inv_freq_inter = (1.0 / (SCALE * pos_freqs)).astype(np.float32)

    def find_dim(num_rot):
        return (HEAD_DIM * math.log(TRAIN_LEN / (num_rot * 2.0 * math.pi))) / (
            2.0 * math.log(THETA)
        )

    low = max(math.floor(find_dim(BETA)), 0)
    high = min(math.ceil(find_dim(ALPHA)), half - 1)
    ramp = np.clip(
        (np.arange(half, dtype=np.float32) - low) / max(high - low, 1e-3), 0.0, 1.0
    ).astype(np.float32)
    extrap = (1.0 - ramp).astype(np.float32)
    inv_freq = inv_freq_inter * (1.0 - extrap) + inv_freq_extra * extrap
    mscale = 0.1 * math.log(SCALE) + 1.0 if SCALE > 1.0 else 1.0
    return inv_freq.astype(np.float32), np.float32(mscale)


def _perm_consts():
    """Permutation lhsT matrices for RoPE on the PE.

    P_rot: within each 64-row block (q rows 0-63, k rows 64-127),
      (P v)[u] = -v[u+32] for u<32, +v[u-32] for u>=32  (rotate-half w/ sign)
    P_swap: (P v)[u] = v[(u+64) % 128]                  (q<->k block swap)
    Matmul computes lhsT.T @ rhs, so pass the TRANSPOSE of each matrix.
    """
    P_rot = np.zeros((128, 128), dtype=np.float32)
    for blk in (0, 64):
        for u in range(32):
            P_rot[blk + u, blk + u + 32] = -1.0
            P_rot[blk + u + 32, blk + u] = 1.0
    P_swap = np.zeros((128, 128), dtype=np.float32)
    for u in range(128):
        P_swap[u, (u + 64) % 128] = 1.0
    P_swrot = P_swap @ P_rot
    ident = np.eye(128, dtype=np.float32)
    # [4, 128, 128]: lhsT variants (transposed), identity last
    perms = np.stack(
        [P_rot.T, P_swrot.T, P_swap.T, ident], axis=0
    )
    return np.ascontiguousarray(perms)


def _tri_consts():
    """fp16 [3, 128, 128]: slot 0 = M^T where M[s,t] = NEG_TRI for s > t
    (strict lower triangle in (s, t)), slot 1 = identity, slot 2 = all
    NEG_TRI (for fully-masked s>t blocks left of the diagonal)."""
    M = np.tril(np.full((128, 128), NEG_TRI, dtype=np.float32), k=-1)
    full = np.full((128, 128), NEG_TRI, dtype=np.float32)
    out = np.stack([M.T, np.eye(128, dtype=np.float32), full], axis=0)
    return np.ascontiguousarray(out.astype(np.float16))


def _host_prep(inputs):
    x = np.asarray(inputs["packed_embeddings"], dtype=np.float32)
    pos = np.asarray(inputs["position_ids"])
    act = np.asarray(inputs["active_mask"])
    wq = np.asarray(inputs["q_proj"], dtype=np.float32)
    wk = np.asarray(inputs["k_proj"], dtype=np.float32)
    wv = np.asarray(inputs["v_proj"], dtype=np.float32)
    wo = np.asarray(inputs["o_proj"], dtype=np.float32)

    inv_freq, mscale = _yarn_inv_freq()
    scale = np.float32(mscale / math.sqrt(HEAD_DIM))

    counts = act.sum(axis=-1)  # (B, HEADS)
    nt_act = max(1, int(-(-counts.max() // 128)))
    na = nt_act * 128
    # round tiles up to a multiple of TPC so chunks are uniform
    nt_act = -(-nt_act // TPC) * TPC
    na = nt_act * 128

    idx = [[np.nonzero(act[b, l])[0] for l in range(HEADS)] for b in range(B)]

    xt = np.zeros((B, HEADS, HIDDEN, na), dtype=np.float16)
    cs = np.zeros((B, HEADS, 2, HEAD_DIM // 2, na), dtype=np.float16)
    a01 = np.zeros((B, HEADS, 1, na), dtype=np.float32)
    for b in range(B):
        for l in range(HEADS):
            ii = idx[b][l]
            n = len(ii)
            xt[b, l, :, :n] = x[b, l, ii, :].T
            ang = pos[b, l, ii].astype(np.float32)[:, None] * inv_freq  # (n, 32)
            cs[b, l, 0, :, :n] = np.cos(ang).T
            cs[b, l, 1, :, :n] = np.sin(ang).T
            a01[b, l, 0, :n] = 1.0

    wqk = np.concatenate([wq * scale, wk], axis=-1)  # (L, 1024, 128)
    wqk16 = np.ascontiguousarray(wqk.astype(np.float16))
    wv16 = np.ascontiguousarray(wv.astype(np.float16))
    wo32 = np.ascontiguousarray(wo)

    perms = _perm_consts()
    tri = _tri_consts()
    meta = {"na": na, "nt_act": nt_act, "counts": counts, "idx": idx}
    return xt, cs, a01, wqk16, wv16, wo32, perms, tri, meta


def _build_program(na):
    nt = na // 128
    nc = bacc.Bacc("TRN2", target_bir_lowering=False, debug=False)

    xt_d = nc.declare_dram_parameter("xt", [B, HPC, HIDDEN, na], F16, isOutput=False)
    cs_d = nc.declare_dram_parameter("cs", [B, HPC, 2, HEAD_DIM // 2, na], F16,
                                     isOutput=False)
    a01_d = nc.declare_dram_parameter("a01", [B, HPC, 1, na], F32, isOutput=False)
    wqk_d = nc.declare_dram_parameter("wqk", [HPC, HIDDEN, 128], F16, isOutput=False)
    wv_d = nc.declare_dram_parameter("wv", [HPC, HIDDEN, HEAD_DIM], F16,
                                     isOutput=False)
    wo_d = nc.declare_dram_parameter("wo", [HPC, HEAD_DIM, HIDDEN], F32R,
                                     isOutput=False)
    perm_d = nc.declare_dram_parameter("perm", [4, 128, 128], F32R, isOutput=False)
    tri_d = nc.declare_dram_parameter("tri", [3, 128, 128], F16, isOutput=False)
    out_d = nc.declare_dram_parameter("out", [B, HPC, na, HIDDEN], F16, isOutput=True)

    with ExitStack() as ctx:
        tc = ctx.enter_context(tile.TileContext(nc))
        _emit(ctx, tc, nc, na, nt, xt_d, cs_d, a01_d, wqk_d, wv_d, wo_d,
              perm_d, tri_d, out_d)
    nc.compile()
    return nc


def _emit(ctx, tc, nc, na, nt, xt_d, cs_d, a01_d, wqk_d, wv_d, wo_d,
          perm_d, tri_d, out_d):
    nchunks = nt // TPC

    # ---- pools ----
    consts = ctx.enter_context(tc.tile_pool(name="consts", bufs=1))
    wpool = ctx.enter_context(tc.tile_pool(name="wpool", bufs=2))
    xtp = ctx.enter_context(tc.tile_pool(name="xt", bufs=2))
    cssp = ctx.enter_context(tc.tile_pool(name="css", bufs=2))
    qkp = ctx.enter_context(tc.tile_pool(name="qk", bufs=2))
    evp = ctx.enter_context(tc.tile_pool(name="ev", bufs=2))
    vtp = ctx.enter_context(tc.tile_pool(name="vt", bufs=2))
    vnp = ctx.enter_context(tc.tile_pool(name="vn", bufs=2))
    ptp = ctx.enter_context(tc.tile_pool(name="pt", bufs=4))
    attp = ctx.enter_context(tc.tile_pool(name="att", bufs=2))
    rap = ctx.enter_context(tc.tile_pool(name="ra", bufs=2))
    outp = ctx.enter_context(tc.tile_pool(name="outsb", bufs=2))

    ps_proj = ctx.enter_context(tc.tile_pool(name="ps_proj", bufs=2, space="PSUM"))
    ps_rk = ctx.enter_context(tc.tile_pool(name="ps_rk", bufs=1, space="PSUM"))
    ps_sc = ctx.enter_context(tc.tile_pool(name="ps_sc", bufs=2, space="PSUM"))
    ps_att = ctx.enter_context(tc.tile_pool(name="ps_att", bufs=1, space="PSUM"))
    ps_o = ctx.enter_context(tc.tile_pool(name="ps_o", bufs=2, space="PSUM"))

    # ---- constants (once) ----
    perm_sb = consts.tile([128, 4, 128], F32R)
    nc.sync.dma_start(out=perm_sb, in_=perm_d.rearrange("k p m -> p k m"))
    tri_sb = consts.tile([128, 3, 128], F16)
    nc.sync.dma_start(out=tri_sb, in_=tri_d.rearrange("k p m -> p k m"))
    ones_sb = consts.tile([128, 1], F32)
    nc.vector.memset(ones_sb, 1.0)

    pairs = [(b, h) for b in range(B) for h in range(HPC)]
    st = {}       # per-pair state
    pending = []  # deferred chunk finishers

    # ---------- phase emitters ----------
    def emit_tables(idx):
        b, h = pairs[idx]
        s = st[idx] = {}
        s["cos"] = cssp.tile([128, na], F16, tag="cos", name="cos_sb")
        base = cs_d[b, h, 0]
        nc.sync.dma_start(out=s["cos"], in_=bass.AP(
            tensor=base.tensor, offset=base.offset, ap=[[0, 4]] + list(base.ap)))
        s["sin"] = cssp.tile([128, na], F16, tag="sin", name="sin_sb")
        base = cs_d[b, h, 1]
        nc.sync.dma_start(out=s["sin"], in_=bass.AP(
            tensor=base.tensor, offset=base.offset, ap=[[0, 4]] + list(base.ap)))
        t_qk = wpool.tile([128, ND, 128], F16, tag="wqk", name="t_qk")
        nc.sync.dma_start(out=t_qk, in_=wqk_d[h].rearrange("(c p) m -> p c m", p=128))
        t_v = wpool.tile([128, ND, HEAD_DIM], F16, tag="wv", name="t_v")
        nc.sync.dma_start(out=t_v, in_=wv_d[h].rearrange("(c p) m -> p c m", p=128))
        t_o = wpool.tile([HEAD_DIM, HIDDEN], F32R, tag="wo", name="t_o")
        nc.sync.dma_start(out=t_o, in_=wo_d[h])
        s["wqk"], s["wv"], s["wo"] = t_qk, t_v, t_o
        # X^T [128, ND, na] fp16, two half-loads for earlier compute start
        s["xt"] = xtp.tile([128, ND, na], F16, tag="xt", name="xt_sb")
        nparts = 4 if idx == 0 else 2
        step = ND // nparts
        src = xt_d[b, h].rearrange("(c p) t -> p c t", p=128)
        for j in range(nparts):
            nc.sync.dma_start(out=s["xt"][:, j * step:(j + 1) * step, :],
                              in_=src[:, j * step:(j + 1) * step, :])
        # vt with active-indicator row 64 (free softmax denominator)
        s["vt"] = vtp.tile([HEAD_DIM + 1, na], F32, tag="vt", name="vt_sb")
        nc.sync.dma_start(out=s["vt"][HEAD_DIM:HEAD_DIM + 1, :],
                          in_=a01_d[b, h])

    def emit_b_steps(idx):
        # projections + RoPE + v for pair idx; generator yields between steps
        s = st[idx]
        xt, wqk, wv = s["xt"], s["wqk"], s["wv"]
        cos, sin = s["cos"], s["sin"]
        qkrot = qkp.tile([128, na], F32R, tag="qkrot", name="qkrot")
        kq = qkp.tile([128, na], F32R, tag="kq", name="kq")
        s["qkrot"], s["kq"] = qkrot, kq
        vt = s["vt"]

        for cx in range(nchunks):
            tsl = slice(cx * CW, (cx + 1) * CW)
            pq = ps_proj.tile([128, CW], F32, tag="proj", name="pq")
            for dc in range(ND):
                nc.tensor.matmul(pq, lhsT=wqk[:, dc, :], rhs=xt[:, dc, tsl],
                                 start=(dc == 0), stop=(dc == ND - 1))
            yield
            ev_c = evp.tile([128, CW], F32R, tag="evc", name="ev_c")
            nc.vector.tensor_mul(ev_c, pq, cos[:, tsl])
            ev_s = evp.tile([128, CW], F32R, tag="evs", name="ev_s")
            nc.vector.tensor_mul(ev_s, pq, sin[:, tsl])
            yield
            pr = ps_rk.tile([128, CW], F32, tag="rk", name="pr")
            nc.tensor.matmul(pr, lhsT=perm_sb[:, 0, :], rhs=ev_s,
                             start=True, stop=False)
            nc.tensor.matmul(pr, lhsT=perm_sb[:, 3, :], rhs=ev_c,
                             start=False, stop=True, skip_group_check=True)
            nc.scalar.copy(qkrot[:, tsl], pr)
            yield
            pk = ps_rk.tile([128, CW], F32, tag="rk", name="pk")
            nc.tensor.matmul(pk, lhsT=perm_sb[:, 1, :], rhs=ev_s,
                             start=True, stop=False)
            nc.tensor.matmul(pk, lhsT=perm_sb[:, 2, :], rhs=ev_c,
                             start=False, stop=True, skip_group_check=True)
            nc.vector.tensor_copy(kq[:, tsl], pk)
            yield
        # v^T pass
        for cx in range(nchunks):
            tsl = slice(cx * CW, (cx + 1) * CW)
            pv = ps_proj.tile([128, CW], F32, tag="proj", name="pv")
            pv64 = pv[0:HEAD_DIM, :]
            for dc in range(ND):
                nc.tensor.matmul(pv64, lhsT=wv[:, dc, :], rhs=xt[:, dc, tsl],
                                 start=(dc == 0), stop=(dc == ND - 1))
            yield
            nc.vector.tensor_copy(vt[0:HEAD_DIM, tsl], pv64)
            yield
        # v natural [s, 65] via PE transposes, packed into 2 psum tiles
        vcols = HEAD_DIM + 1
        vn = vnp.tile([128, nt * vcols], BF16, tag="vn", name="vn")
        s["vn"] = vn
        groups = [(0, 5), (5, nt)] if nt > 5 else [(0, nt)]
        for g0, g1 in groups:
            pvt = ps_rk.tile([128, CW], F32, tag="rk", name="pvt")
            for si in range(g0, g1):
                nc.tensor.transpose(
                    out=pvt[:, (si - g0) * vcols:(si - g0 + 1) * vcols],
                    in_=vt[:, si * 128:(si + 1) * 128],
                    identity=perm_sb[0:vcols, 3, 0:vcols].bitcast(F32),
                )
            nc.vector.tensor_copy(
                vn[:, g0 * vcols:g1 * vcols],
                pvt[:, 0:(g1 - g0) * vcols])
            yield

    def make_finisher(idx, cx, att_sb, ra):
        b, h = pairs[idx]
        s = st[idx]
        wo = s["wo"]

        def fin():
            for k in range(TPC):
                ti = cx * TPC + k
                osb = outp.tile([128, HIDDEN], F16, tag="osb", name="osb")
                for dh in range(2):
                    po = ps_o.tile([128, 512], F32, tag="o", name="po")
                    nc.tensor.matmul(
                        po,
                        lhsT=att_sb[0:HEAD_DIM, k * 128:(k + 1) * 128],
                        rhs=wo[:, dh * 512:(dh + 1) * 512],
                        start=True, stop=True,
                    )
                    dst = osb[:, dh * 512:(dh + 1) * 512]
                    if (k * 2 + dh) % 2 == 0 and k == 1:
                        nc.scalar.mul(dst, po, ra[:, k:k + 1])
                    else:
                        nc.vector.tensor_scalar_mul(dst, po, ra[:, k:k + 1])
                    yield
                nc.sync.dma_start(
                    out=out_d[b, h, ti * 128:(ti + 1) * 128, :], in_=osb)
        return fin()

    def step_pending():
        if pending:
            if next(pending[0], StopIteration) is StopIteration:
                pending.pop(0)

    def drain_oldest():
        if pending:
            gen = pending.pop(0)
            for _ in gen:
                pass

    def emit_c_chunk(idx, cx, filler=None):
        s = st[idx]
        qkrot, kq, vn = s["qkrot"], s["kq"], s["vn"]
        tsl = slice(cx * CW, (cx + 1) * CW)
        n_s = TPC * (cx + 1)
        while len(pending) > 1:
            drain_oldest()
        patt = ps_att.tile([HEAD_DIM + 1, CW], F32, tag="att", name="patt")
        prob_tiles = []
        vcols = HEAD_DIM + 1

        def att_mm(si):
            kd = si - TPC * cx
            lo = kd * 128 if kd > 0 else 0
            nc.tensor.matmul(patt[:, lo:], lhsT=vn[:, si * vcols:(si + 1) * vcols],
                             rhs=prob_tiles[si][:, lo:],
                             start=(si == 0), stop=(si == n_s - 1),
                             skip_group_check=True)

        for si in range(n_s):
            psc = ps_sc.tile([128, CW], F32, tag="sc", name="psc")
            kd = si - TPC * cx
            diag = kd >= 0
            if si % 2 == 0:
                nc.tensor.matmul(
                    psc,
                    lhsT=kq[0:HEAD_DIM, si * 128:(si + 1) * 128],
                    rhs=qkrot[0:HEAD_DIM, tsl],
                    start=True, stop=not diag,
                )
            else:
                nc.tensor.matmul(
                    psc,
                    lhsT=qkrot[HEAD_DIM:128, si * 128:(si + 1) * 128],
                    rhs=kq[HEAD_DIM:128, tsl],
                    start=True, stop=not diag,
                )
            if diag:
                nc.tensor.matmul(
                    psc[:, kd * 128:(kd + 1) * 128],
                    lhsT=tri_sb[:, 0, :], rhs=tri_sb[:, 1, :],
                    start=False, stop=True, skip_group_check=True,
                )
            pt = ptp.tile([128, CW], BF16, tag="pt", name="pt")
            if diag and kd > 0:
                # columns left of the diagonal block are fully masked (s > t):
                # zero them on the (otherwise idle) Pool engine and exp the rest
                nc.gpsimd.memset(pt[:, 0:kd * 128], 0.0)
                nc.scalar.activation(pt[:, kd * 128:], psc[:, kd * 128:],
                                     mybir.ActivationFunctionType.Exp)
            else:
                nc.scalar.activation(pt, psc, mybir.ActivationFunctionType.Exp)
            prob_tiles.append(pt)
            if filler is not None:
                next(filler, None)
            step_pending()
            if si >= 2:
                att_mm(si - 2)
        att_mm(n_s - 2)
        att_mm(n_s - 1)

        att_sb = attp.tile([HEAD_DIM + 1, CW], F32R, tag="attsb", name="att_sb")
        if cx % 2 == 0:
            nc.scalar.copy(att_sb, patt)
        else:
            nc.vector.tensor_copy(att_sb, patt)
        # denominator: PE-transpose row 64 to columns, reciprocal (tiny ops)
        pdn = ps_sc.tile([128, CW], F32, tag="sc", name="pdn")
        for k in range(TPC):
            nc.tensor.transpose(
                out=pdn[:, k:k + 1],
                in_=att_sb[HEAD_DIM:HEAD_DIM + 1,
                           k * 128:(k + 1) * 128].bitcast(F32),
                identity=ones_sb[HEAD_DIM:HEAD_DIM + 1, :],
            )
        ra = rap.tile([128, TPC], F32, tag="ra", name="ra")
        nc.vector.reciprocal(ra, pdn[:, 0:TPC])

        pending.append(make_finisher(idx, cx, att_sb, ra))

    # ---------- interleaved pipeline across pairs ----------
    emit_tables(0)
    for _ in emit_b_steps(0):
        pass
    for idx in range(NPAIRS):
        filler = None
        if idx + 1 < NPAIRS:
            emit_tables(idx + 1)
            filler = emit_b_steps(idx + 1)
        for cx in range(nchunks):
            emit_c_chunk(idx, cx, filler)
        if filler is not None:
            for _ in filler:
                pass
        if idx > 0:
            del st[idx - 1]
    while pending:
        drain_oldest()


_PROGRAM = {}


def _prep_in_maps(inputs):
    xt, cs, a01, wqk16, wv16, wo32, perms, tri, meta = _host_prep(inputs)
    in_maps = []
    for c in range(NCORES):
        hs = slice(c * HPC, (c + 1) * HPC)
        in_maps.append({
            "xt": np.ascontiguousarray(xt[:, hs]),
            "cs": np.ascontiguousarray(cs[:, hs]),
            "a01": np.ascontiguousarray(a01[:, hs]),
            "wqk": np.ascontiguousarray(wqk16[hs]),
            "wv": np.ascontiguousarray(wv16[hs]),
            "wo": np.ascontiguousarray(wo32[hs]),
            "perm": perms,
            "tri": tri,
        })
    return in_maps, meta


def kernel(**inputs) -> np.ndarray:
    in_maps, meta = _prep_in_maps(inputs)
    na = meta["na"]

    if na not in _PROGRAM:
        _PROGRAM[na] = _build_program(na)
    nc = _PROGRAM[na]

    res = run_bass_kernel_spmd(nc, in_maps, list(range(NCORES)))

    out = np.zeros((B, HEADS, T, HIDDEN), dtype=np.float32)
    idx = meta["idx"]
    for c in range(NCORES):
        oc = res.results[c]["out"]  # [B, HPC, na, HIDDEN] fp16
        for b in range(B):
            for hh in range(HPC):
                l = c * HPC + hh
                ii = idx[b][l]
                out[b, l, ii, :] = oc[b, hh, :len(ii), :].astype(np.float32)
    return out


# revision 20
# speedup vs baseline: 2.8411x; 1.0263x over previous
"""Trainium2 Bass kernel for BottleneckedEnsembleAttention (sparse/compacted).

Sharding: 8 cores, core c handles heads [2c, 2c+1] for both batches
(4 independent (b, head) attention problems per core).

Sparsity: the reference zeroes output rows for inactive queries, masks
inactive keys out of the softmax, and inactive tokens never otherwise
contribute.  The host therefore COMPACTS each (b, h) problem to its active
tokens (order-preserving, so the causal mask stays lower-triangular), pads
to NA = ceil(max_active/128)*128, and scatters the device output back into
a zero tensor.  Seed-0 counts are ~1024 of 2048, so NA = 1152: projections
shrink ~2x and attention area ~3.2x.

Host also pre-transposes the compacted X to [HIDDEN, NA] fp16 (no on-device
transposes for X), folds the softmax scale into Wq, and computes compacted
YaRN cos/sin tables [32, NA] fp16 (rows are 32-periodic on device).

Per (b, h) on-device pipeline (all heavy matmuls 1 cycle/col):
  1. qk pass: psum_qk = [Wq*scale | Wk]^T X^T  -> [128, ch] (q^T rows 0-63,
     k^T rows 64-127), fp16 inputs.
  2. RoPE on PE: ev_c = psum_qk * cos, ev_s = psum_qk * sin (DVE);
     cos/sin rows are 32-periodic so rotate-half/q-k-swap permutations
     commute with the elementwise multiplies:
       qkrot = P_rot @ ev_s + ev_c          (2 matmuls into one psum)
       kq    = (Psw P_rot) @ ev_s + Psw @ ev_c   (k in rows 0-63, q in 64-127)
  3. v^T pass (fp16) -> vt [65, NA] with row 64 = active-indicator (for the
     free softmax denominator), PE-transposed to vn [s, 65].
  4. per t-chunk (384 cols): scores^T[s-tile, t] = k lhsT @ q rhs; causal
     mask added INSIDE the matmul via an fp16 strict-upper -60000 triangular
     lhsT against an identity rhs on the diagonal 128-block; exp via ACT;
     att^T[u, t] accumulated over s-tiles (row 64 = denominator);
     o_proj: out[t-tile, 1024] = att^T lhsT @ Wo rhs, scaled by 1/denom at
     PSUM eviction (denom reciprocal as a row, PE-transposed to columns).
  5. store compacted fp16 output rows; host upcasts and scatters.

The emission is software-pipelined across the 4 (b, h) problems: the next
pair's projections/RoPE/v are interleaved as PE gap-filler inside the
current pair's attention chunks, and each chunk's o_proj/store is deferred
one chunk so the PE never waits on eviction chains.  PSUM evictions are
spread across ACT/DVE/Pool engines.
"""

import math
from contextlib import ExitStack

import numpy as np

import concourse.bass as bass
import concourse.mybir as mybir
import concourse.tile as tile
from concourse import bacc
from concourse.bass_utils import run_bass_kernel_spmd

# model constants (must match reference.py)
HIDDEN = 1024
HEADS = 16
HEAD_DIM = 64
THETA = 10000.0
TRAIN_LEN = 2048
SCALE = 4.0
ALPHA = 1.0
BETA = 32.0
B, T = 2, 2048

NCORES = 8
HPC = HEADS // NCORES  # heads per core = 2
NPAIRS = B * HPC       # independent (b, h) problems per core = 4

F32 = mybir.dt.float32
F32R = mybir.dt.float32r
F16 = mybir.dt.float16
BF16 = mybir.dt.bfloat16

ND = HIDDEN // 128  # 8 d-chunks
CW = 384            # chunk width (3 t-tiles)
TPC = CW // 128     # t-tiles per chunk = 3
NEG_TRI = -60000.0  # fp16-representable; exp(score + NEG_TRI) == 0.0


def _yarn_inv_freq():
    half = HEAD_DIM // 2
    pos_freqs = THETA ** (np.arange(half, dtype=np.float32) * 2.0 / HEAD_DIM)
    inv_freq_extra = (1.0 / pos_freqs).astype(np.float32)
    inv_freq_inter = (1.0 / (SCALE * pos_freqs)).astype(np.float32)

    def find_dim(num_rot):
        return (HEAD_DIM * math.log(TRAIN_LEN / (num_rot * 2.0 * math.pi))) / (
            2.0 * math.log(THETA)
        )

    low = max(math.floor(find_dim(BETA)), 0)
    high = min(math.ceil(find_dim(ALPHA)), half - 1)
    ramp = np.clip(
        (np.arange(half, dtype=np.float32) - low) / max(high - low, 1e-3), 0.0, 1.0
    ).astype(np.float32)
    extrap = (1.0 - ramp).astype(np.float32)
    inv_freq = inv_freq_inter * (1.0 - extrap) + inv_freq_extra * extrap
    mscale = 0.1 * math.log(SCALE) + 1.0 if SCALE > 1.0 else 1.0
    return inv_freq.astype(np.float32), np.float32(mscale)


def _perm_consts():
    """Permutation lhsT matrices for RoPE on the PE.

    P_rot: within each 64-row block (q rows 0-63, k rows 64-127),
      (P v)[u] = -v[u+32] for u<32, +v[u-32] for u>=32  (rotate-half w/ sign)
    P_swap: (P v)[u] = v[(u+64) % 128]                  (q<->k block swap)
    Matmul computes lhsT.T @ rhs, so pass the TRANSPOSE of each matrix.
    """
    P_rot = np.zeros((128, 128), dtype=np.float32)
    for blk in (0, 64):
        for u in range(32):
            P_rot[blk + u, blk + u + 32] = -1.0
            P_rot[blk + u + 32, blk + u] = 1.0
    P_swap = np.zeros((128, 128), dtype=np.float32)
    for u in range(128):
        P_swap[u, (u + 64) % 128] = 1.0
    P_swrot = P_swap @ P_rot
    ident = np.eye(128, dtype=np.float32)
    # [4, 128, 128]: lhsT variants (transposed), identity last
    perms = np.stack(
        [P_rot.T, P_swrot.T, P_swap.T, ident], axis=0
    )
    return np.ascontiguousarray(perms)


def _tri_consts():
    """fp16 [3, 128, 128]: slot 0 = M^T where M[s,t] = NEG_TRI for s > t
    (strict lower triangle in (s, t)), slot 1 = identity, slot 2 = all
    NEG_TRI (for fully-masked s>t blocks left of the diagonal)."""
    M = np.tril(np.full((128, 128), NEG_TRI, dtype=np.float32), k=-1)
    full = np.full((128, 128), NEG_TRI, dtype=np.float32)
    out = np.stack([M.T, np.eye(128, dtype=np.float32), full], axis=0)
    return np.ascontiguousarray(out.astype(np.float16))


def _host_prep(inputs):
    x = np.asarray(inputs["packed_embeddings"], dtype=np.float32)
    pos = np.asarray(inputs["position_ids"])
    act = np.asarray(inputs["active_mask"])
    wq = np.asarray(inputs["q_proj"], dtype=np.float32)
    wk = np.asarray(inputs["k_proj"], dtype=np.float32)
    wv = np.asarray(inputs["v_proj"], dtype=np.float32)
    wo = np.asarray(inputs["o_proj"], dtype=np.float32)

    inv_freq, mscale = _yarn_inv_freq()
    scale = np.float32(mscale / math.sqrt(HEAD_DIM))

    counts = act.sum(axis=-1)  # (B, HEADS)
    nt_act = max(1, int(-(-counts.max() // 128)))
    na = nt_act * 128
    # round tiles up to a multiple of TPC so chunks are uniform
    nt_act = -(-nt_act // TPC) * TPC
    na = nt_act * 128

    idx = [[np.nonzero(act[b, l])[0] for l in range(HEADS)] for b in range(B)]

    xt = np.zeros((B, HEADS, HIDDEN, na), dtype=np.float16)
    cs = np.zeros((B, HEADS, 2, HEAD_DIM // 2, na), dtype=np.float16)
    a01 = np.zeros((B, HEADS, 1, na), dtype=np.float32)
    for b in range(B):
        for l in range(HEADS):
            ii = idx[b][l]
            n = len(ii)
            xt[b, l, :, :n] = x[b, l, ii, :].T
            ang = pos[b, l, ii].astype(np.float32)[:, None] * inv_freq  # (n, 32)
            cs[b, l, 0, :, :n] = np.cos(ang).T
            cs[b, l, 1, :, :n] = np.sin(ang).T
            a01[b, l, 0, :n] = 1.0

    wqk = np.concatenate([wq * scale, wk], axis=-1)  # (L, 1024, 128)
    wqk16 = np.ascontiguousarray(wqk.astype(np.float16))
    wv16 = np.ascontiguousarray(wv.astype(np.float16))
    wo32 = np.ascontiguousarray(wo)

    perms = _perm_consts()
    tri = _tri_consts()
    meta = {"na": na, "nt_act": nt_act, "counts": counts, "idx": idx}
    return xt, cs, a01, wqk16, wv16, wo32, perms, tri, meta


def _build_program(na):
    nt = na // 128
    nc = bacc.Bacc("TRN2", target_bir_lowering=False, debug=False)

    xt_d = nc.declare_dram_parameter("xt", [B, HPC, HIDDEN, na], F16, isOutput=False)
    cs_d = nc.declare_dram_parameter("cs", [B, HPC, 2, HEAD_DIM // 2, na], F16,
                                     isOutput=False)
    a01_d = nc.declare_dram_parameter("a01", [B, HPC, 1, na], F32, isOutput=False)
    wqk_d = nc.declare_dram_parameter("wqk", [HPC, HIDDEN, 128], F16, isOutput=False)
    wv_d = nc.declare_dram_parameter("wv", [HPC, HIDDEN, HEAD_DIM], F16,
                                     isOutput=False)
    wo_d = nc.declare_dram_parameter("wo", [HPC, HEAD_DIM, HIDDEN], F32R,
                                     isOutput=False)
    perm_d = nc.declare_dram_parameter("perm", [4, 128, 128], F32R, isOutput=False)
    tri_d = nc.declare_dram_parameter("tri", [3, 128, 128], F16, isOutput=False)
    out_d = nc.declare_dram_parameter("out", [B, HPC, na, HIDDEN], F16, isOutput=True)

    with ExitStack() as ctx:
        tc = ctx.enter_context(tile.TileContext(nc))
        _emit(ctx, tc, nc, na, nt, xt_d, cs_d, a01_d, wqk_d, wv_d, wo_d,
              perm_d, tri_d, out_d)
    nc.compile()
    return nc


def _emit(ctx, tc, nc, na, nt, xt_d, cs_d, a01_d, wqk_d, wv_d, wo_d,
          perm_d, tri_d, out_d):
    nchunks = nt // TPC

    # ---- pools ----
    consts = ctx.enter_context(tc.tile_pool(name="consts", bufs=1))
    wpool = ctx.enter_context(tc.tile_pool(name="wpool", bufs=2))
    xtp = ctx.enter_context(tc.tile_pool(name="xt", bufs=2))
    cssp = ctx.enter_context(tc.tile_pool(name="css", bufs=2))
    qkp = ctx.enter_context(tc.tile_pool(name="qk", bufs=2))
    evp = ctx.enter_context(tc.tile_pool(name="ev", bufs=2))
    vtp = ctx.enter_context(tc.tile_pool(name="vt", bufs=2))
    vnp = ctx.enter_context(tc.tile_pool(name="vn", bufs=2))
    ptp = ctx.enter_context(tc.tile_pool(name="pt", bufs=4))
    attp = ctx.enter_context(tc.tile_pool(name="att", bufs=2))
    rap = ctx.enter_context(tc.tile_pool(name="ra", bufs=2))
    outp = ctx.enter_context(tc.tile_pool(name="outsb", bufs=2))

    ps_proj = ctx.enter_context(tc.tile_pool(name="ps_proj", bufs=2, space="PSUM"))
    ps_rk = ctx.enter_context(tc.tile_pool(name="ps_rk", bufs=1, space="PSUM"))
    ps_sc = ctx.enter_context(tc.tile_pool(name="ps_sc", bufs=2, space="PSUM"))
    ps_att = ctx.enter_context(tc.tile_pool(name="ps_att", bufs=1, space="PSUM"))
    ps_o = ctx.enter_context(tc.tile_pool(name="ps_o", bufs=2, space="PSUM"))

    # ---- constants (once) ----
    perm_sb = consts.tile([128, 4, 128], F32R)
    nc.sync.dma_start(out=perm_sb, in_=perm_d.rearrange("k p m -> p k m"))
    tri_sb = consts.tile([128, 3, 128], F16)
    nc.sync.dma_start(out=tri_sb, in_=tri_d.rearrange("k p m -> p k m"))
    ones_sb = consts.tile([128, 1], F32)
    nc.vector.memset(ones_sb, 1.0)

    pairs = [(b, h) for b in range(B) for h in range(HPC)]
    st = {}       # per-pair state
    pending = []  # deferred chunk finishers

    # ---------- phase emitters ----------
    def emit_tables(idx):
        b, h = pairs[idx]
        s = st[idx] = {}
        t_qk = wpool.tile([128, ND, 128], F16, tag="wqk", name="t_qk")
        nc.sync.dma_start(out=t_qk, in_=wqk_d[h].rearrange("(c p) m -> p c m", p=128))
        s["xt"] = xtp.tile([128, ND, na], F16, tag="xt", name="xt_sb")
        nparts = 4 if idx == 0 else 2
        step = ND // nparts
        src = xt_d[b, h].rearrange("(c p) t -> p c t", p=128)
        for j in range(nparts):
            nc.sync.dma_start(out=s["xt"][:, j * step:(j + 1) * step, :],
                              in_=src[:, j * step:(j + 1) * step, :])
        s["cos"] = cssp.tile([128, na], F16, tag="cos", name="cos_sb")
        base = cs_d[b, h, 0]
        nc.sync.dma_start(out=s["cos"], in_=bass.AP(
            tensor=base.tensor, offset=base.offset, ap=[[0, 4]] + list(base.ap)))
        s["sin"] = cssp.tile([128, na], F16, tag="sin", name="sin_sb")
        base = cs_d[b, h, 1]
        nc.sync.dma_start(out=s["sin"], in_=bass.AP(
            tensor=base.tensor, offset=base.offset, ap=[[0, 4]] + list(base.ap)))
        t_v = wpool.tile([128, ND, HEAD_DIM], F16, tag="wv", name="t_v")
        nc.sync.dma_start(out=t_v, in_=wv_d[h].rearrange("(c p) m -> p c m", p=128))
        t_o = wpool.tile([HEAD_DIM, HIDDEN], F32R, tag="wo", name="t_o")
        nc.sync.dma_start(out=t_o, in_=wo_d[h])
        s["wqk"], s["wv"], s["wo"] = t_qk, t_v, t_o
        # vt with active-indicator row 64 (free softmax denominator)
        s["vt"] = vtp.tile([HEAD_DIM + 1, na], F32, tag="vt", name="vt_sb")
        nc.sync.dma_start(out=s["vt"][HEAD_DIM:HEAD_DIM + 1, :],
                          in_=a01_d[b, h])

    def emit_b_steps(idx):
        # projections + RoPE + v for pair idx; generator yields between steps
        s = st[idx]
        xt, wqk, wv = s["xt"], s["wqk"], s["wv"]
        cos, sin = s["cos"], s["sin"]
        qkrot = qkp.tile([128, na], F32R, tag="qkrot", name="qkrot")
        kq = qkp.tile([128, na], F32R, tag="kq", name="kq")
        s["qkrot"], s["kq"] = qkrot, kq
        vt = s["vt"]

        for cx in range(nchunks):
            tsl = slice(cx * CW, (cx + 1) * CW)
            pq = ps_proj.tile([128, CW], F32, tag="proj", name="pq")
            for dc in range(ND):
                nc.tensor.matmul(pq, lhsT=wqk[:, dc, :], rhs=xt[:, dc, tsl],
                                 start=(dc == 0), stop=(dc == ND - 1))
            yield
            ev_c = evp.tile([128, CW], F32R, tag="evc", name="ev_c")
            nc.vector.tensor_mul(ev_c, pq, cos[:, tsl])
            ev_s = evp.tile([128, CW], F32R, tag="evs", name="ev_s")
            nc.vector.tensor_mul(ev_s, pq, sin[:, tsl])
            yield
            pr = ps_rk.tile([128, CW], F32, tag="rk", name="pr")
            nc.tensor.matmul(pr, lhsT=perm_sb[:, 0, :], rhs=ev_s,
                             start=True, stop=False)
            nc.tensor.matmul(pr, lhsT=perm_sb[:, 3, :], rhs=ev_c,
                             start=False, stop=True, skip_group_check=True)
            nc.scalar.copy(qkrot[:, tsl], pr)
            yield
            pk = ps_rk.tile([128, CW], F32, tag="rk", name="pk")
            nc.tensor.matmul(pk, lhsT=perm_sb[:, 1, :], rhs=ev_s,
                             start=True, stop=False)
            nc.tensor.matmul(pk, lhsT=perm_sb[:, 2, :], rhs=ev_c,
                             start=False, stop=True, skip_group_check=True)
            nc.vector.tensor_copy(kq[:, tsl], pk)
            yield
        # v^T pass
        for cx in range(nchunks):
            tsl = slice(cx * CW, (cx + 1) * CW)
            pv = ps_proj.tile([128, CW], F32, tag="proj", name="pv")
            pv64 = pv[0:HEAD_DIM, :]
            for dc in range(ND):
                nc.tensor.matmul(pv64, lhsT=wv[:, dc, :], rhs=xt[:, dc, tsl],
                                 start=(dc == 0), stop=(dc == ND - 1))
            yield
            nc.vector.tensor_copy(vt[0:HEAD_DIM, tsl], pv64)
            yield
        # v natural [s, 65] via PE transposes, packed into 2 psum tiles
        vcols = HEAD_DIM + 1
        vn = vnp.tile([128, nt * vcols], BF16, tag="vn", name="vn")
        s["vn"] = vn
        groups = [(0, 5), (5, nt)] if nt > 5 else [(0, nt)]
        for g0, g1 in groups:
            pvt = ps_rk.tile([128, CW], F32, tag="rk", name="pvt")
            for si in range(g0, g1):
                nc.tensor.transpose(
                    out=pvt[:, (si - g0) * vcols:(si - g0 + 1) * vcols],
                    in_=vt[:, si * 128:(si + 1) * 128],
                    identity=perm_sb[0:vcols, 3, 0:vcols].bitcast(F32),
                )
            nc.vector.tensor_copy(
                vn[:, g0 * vcols:g1 * vcols],
                pvt[:, 0:(g1 - g0) * vcols])
            yield

    def make_finisher(idx, cx, att_sb, ra):
        b, h = pairs[idx]
        s = st[idx]
        wo = s["wo"]

        def fin():
            for k in range(TPC):
                ti = cx * TPC + k
                osb = outp.tile([128, HIDDEN], F16, tag="osb", name="osb")
                for dh in range(2):
                    po = ps_o.tile([128, 512], F32, tag="o", name="po")
                    nc.tensor.matmul(
                        po,
                        lhsT=att_sb[0:HEAD_DIM, k * 128:(k + 1) * 128],
                        rhs=wo[:, dh * 512:(dh + 1) * 512],
                        start=True, stop=True,
                    )
                    dst = osb[:, dh * 512:(dh + 1) * 512]
                    if (k * 2 + dh) % 2 == 0:
                        nc.scalar.mul(dst, po, ra[:, k:k + 1])
                    else:
                        nc.vector.tensor_scalar_mul(dst, po, ra[:, k:k + 1])
                    yield
                nc.sync.dma_start(
                    out=out_d[b, h, ti * 128:(ti + 1) * 128, :], in_=osb)
        return fin()

    def step_pending():
        if pending:
            if next(pending[0], StopIteration) is StopIteration:
                pending.pop(0)

    def drain_oldest():
        if pending:
            gen = pending.pop(0)
            for _ in gen:
                pass

    def emit_c_chunk(idx, cx, filler=None):
        s = st[idx]
        qkrot, kq, vn = s["qkrot"], s["kq"], s["vn"]
        tsl = slice(cx * CW, (cx + 1) * CW)
        n_s = TPC * (cx + 1)
        while len(pending) > 1:
            drain_oldest()
        patt = ps_att.tile([HEAD_DIM + 1, CW], F32, tag="att", name="patt")
        prob_tiles = []
        vcols = HEAD_DIM + 1

        def att_mm(si):
            kd = si - TPC * cx
            lo = kd * 128 if kd > 0 else 0
            nc.tensor.matmul(patt[:, lo:], lhsT=vn[:, si * vcols:(si + 1) * vcols],
                             rhs=prob_tiles[si][:, lo:],
                             start=(si == 0), stop=(si == n_s - 1),
                             skip_group_check=True)

        for si in range(n_s):
            psc = ps_sc.tile([128, CW], F32, tag="sc", name="psc")
            kd = si - TPC * cx
            diag = kd >= 0
            lo = 128 if kd >= 1 else 0
            tslo = slice(cx * CW + lo, (cx + 1) * CW)
            if si % 2 == 0:
                nc.tensor.matmul(
                    psc[:, lo:],
                    lhsT=kq[0:HEAD_DIM, si * 128:(si + 1) * 128],
                    rhs=qkrot[0:HEAD_DIM, tslo],
                    start=True, stop=not diag,
                )
            else:
                nc.tensor.matmul(
                    psc[:, lo:],
                    lhsT=qkrot[HEAD_DIM:128, si * 128:(si + 1) * 128],
                    rhs=kq[HEAD_DIM:128, tslo],
                    start=True, stop=not diag,
                )
            if diag:
                nc.tensor.matmul(
                    psc[:, kd * 128:(kd + 1) * 128],
                    lhsT=tri_sb[:, 0, :], rhs=tri_sb[:, 1, :],
                    start=False, stop=True, skip_group_check=True,
                )
            pt = ptp.tile([128, CW], BF16, tag="pt", name="pt")
            if diag and kd > 0:
                # columns left of the diagonal block are fully masked (s > t):
                # zero them on the (otherwise idle) Pool engine and exp the rest
                nc.gpsimd.memset(pt[:, 0:kd * 128], 0.0)
                nc.scalar.activation(pt[:, kd * 128:], psc[:, kd * 128:],
                                     mybir.ActivationFunctionType.Exp)
            else:
                nc.scalar.activation(pt, psc, mybir.ActivationFunctionType.Exp)
            prob_tiles.append(pt)
            if filler is not None:
                next(filler, None)
            step_pending()
            if si >= 2:
                att_mm(si - 2)
        att_mm(n_s - 2)
        att_mm(n_s - 1)

        att_sb = attp.tile([HEAD_DIM + 1, CW], F32R, tag="attsb", name="att_sb")
        if cx % 2 == 0:
            nc.scalar.copy(att_sb, patt)
        else:
            nc.vector.tensor_copy(att_sb, patt)
        # denominator: PE-transpose row 64 to columns, reciprocal (tiny ops)
        pdn = ps_o.tile([128, 512], F32, tag="o", name="pdn")
        for k in range(TPC):
            nc.tensor.transpose(
                out=pdn[:, k:k + 1],
                in_=att_sb[HEAD_DIM:HEAD_DIM + 1,
                           k * 128:(k + 1) * 128].bitcast(F32),
                identity=ones_sb[HEAD_DIM:HEAD_DIM + 1, :],
            )
        ra = rap.tile([128, TPC], F32, tag="ra", name="ra")
        nc.vector.reciprocal(ra, pdn[:, 0:TPC])

        pending.append(make_finisher(idx, cx, att_sb, ra))

    # ---------- interleaved pipeline across pairs ----------
    emit_tables(0)
    for _ in emit_b_steps(0):
        pass
    for idx in range(NPAIRS):
        filler = None
        if idx + 1 < NPAIRS:
            emit_tables(idx + 1)
            filler = emit_b_steps(idx + 1)
        for cx in range(nchunks):
            emit_c_chunk(idx, cx, filler)
        if filler is not None:
            for _ in filler:
                pass
        if idx > 0:
            del st[idx - 1]
    while pending:
        drain_oldest()


_PROGRAM = {}


def _prep_in_maps(inputs):
    xt, cs, a01, wqk16, wv16, wo32, perms, tri, meta = _host_prep(inputs)
    in_maps = []
    for c in range(NCORES):
        hs = slice(c * HPC, (c + 1) * HPC)
        in_maps.append({
            "xt": np.ascontiguousarray(xt[:, hs]),
            "cs": np.ascontiguousarray(cs[:, hs]),
            "a01": np.ascontiguousarray(a01[:, hs]),
            "wqk": np.ascontiguousarray(wqk16[hs]),
            "wv": np.ascontiguousarray(wv16[hs]),
            "wo": np.ascontiguousarray(wo32[hs]),
            "perm": perms,
            "tri": tri,
        })
    return in_maps, meta


def kernel(**inputs) -> np.ndarray:
    in_maps, meta = _prep_in_maps(inputs)
    na = meta["na"]

    if na not in _PROGRAM:
        _PROGRAM[na] = _build_program(na)
    nc = _PROGRAM[na]

    res = run_bass_kernel_spmd(nc, in_maps, list(range(NCORES)))

    out = np.zeros((B, HEADS, T, HIDDEN), dtype=np.float32)
    idx = meta["idx"]
    for c in range(NCORES):
        oc = res.results[c]["out"]  # [B, HPC, na, HIDDEN] fp16
        for b in range(B):
            for hh in range(HPC):
                l = c * HPC + hh
                ii = idx[b][l]
                out[b, l, ii, :] = oc[b, hh, :len(ii), :].astype(np.float32)
    return out
